# revision 78
# baseline (speedup 1.0000x reference)
"""Distributed causal-attention-with-dropout kernel for 8 TRN2 NeuronCores, v19.

Architecture ("all-local projections", fully static SPMD graph):

- Host pre-formats inputs (layout only, all model FLOPs stay on device):
  each core receives xq = x^T columns of its 4 OWNED q-tiles
  {c, 15-c, 16+c, 31-c} (bf16), the FULL Wq^T / Wk^T / Wv^T (bf16,
  replicated), dropout-mask rows of its owned tiles (bf16), and the causal
  threshold table.  There is NO x gather at all.
- A tiny dummy AllGather with NO input dependency fires at t~0 so the
  collectives-runtime init barrier (~20+40us) overlaps the K projection.
- Tensor phase order: K proj -> V slots {0,1} -> V slots {2,3} -> Q proj
  -> attention.  Startup loads are interleaved (wk ki-chunk, xq ki-chunk)
  across all 3 DMA queues and the K projection consumes ki in arrival
  order, so the PE starts at ~4us and stays dense (HAM un-throttled).
- K^T is AllGathered at FULL d_out depth in k-tile-group chunks, with
  group 0 further split into d_out halves so the first AG fires mid-K-proj:
  CC chain = KAG0a (4MB, after dp3), KAG0b (4MB), VAG0, KAG1 (8MB),
  VAG1 — exactly the attention consumption order, with no CC idle.
- Score big-block B needs only kg group B//2; block tiles share slot
  parity so each block reads one 128-column stripe of the AG output.
- kt tiles for blocks 0/1 load DURING the Q projection (the wv weight
  pool closes after V, freeing SBUF for ktl).  The KAG1/VAG1 triggers are
  EMITTED after the B=0/1 loads+pairs: DRAM DMAs conservatively wait on
  in-flight collectives, and CC-completion-dependent DMAs must NEVER sit
  on the gpsimd queue (they'd block the later CC triggers behind them).
- Attention: k-blocks are 1024 wide -> ZERO-padding static schedule
  (slot s needs exactly s+1 blocks; 10 pairs).  Causality enforced
  per-row by (iota(p-j) >= thr) * P on the vector engine; softmax without
  max-subtraction; denominators use pre-dropout sums.  Pair p's
  P-transposes and attn@V run after pair p+1's score matmuls.
"""

import math
import os
import sys
from contextlib import ExitStack

import numpy as np
import ml_dtypes

for _p in ("/opt/trn_rl_repo", "/root/.axon_site/_ro/trn_rl_repo"):
    if os.path.isdir(_p) and _p not in sys.path:
        sys.path.append(_p)

import concourse.bass as bass
import concourse.tile as tile
from concourse import bacc, mybir
from concourse import bass_utils
from concourse.masks import make_identity

S, D = 4096, 2048
NC = 8
SB = 512          # seq rows per core (4 owned 128-tiles)
BK = 1024         # big k-block width
NBIG = 4
KBMAX = (1, 2, 3, 4)
PBASE = (0, 1, 3, 6)
NPAIR = 10
SCALE = 1.0 / math.sqrt(float(D))
F32 = mybir.dt.float32
BF16 = mybir.dt.bfloat16
RG = [list(range(NC))]
ALU = mybir.AluOpType
AFT = mybir.ActivationFunctionType

# ki consumption order for the K projection: matches the 3-queue load
# arrival pattern (sync: ki 0..4, gpsimd: ki 5..9, scalar: ki 10..15) so
# the first dp group issues matmuls as chunks land.
KI_ORDER = (0, 5, 10, 1, 6, 11, 2, 7, 12, 3, 8, 13, 4, 9, 14, 15)


def owned_tiles(c):
    return (c, 15 - c, 16 + c, 31 - c)


def tile_owner_slot(t):
    if t <= 7:
        return t, 0
    if t <= 15:
        return 15 - t, 1
    if t <= 23:
        return t - 16, 2
    return 31 - t, 3


# row of tile t inside its V AllGather chunk (chunk = t//16; within a
# chunk, rank blocks of 256 rows hold slots {0,1} or {2,3})
VROW2 = [256 * tile_owner_slot(t)[0] + 128 * (tile_owner_slot(t)[1] % 2)
         for t in range(32)]


def build():
    nc = bacc.Bacc("TRN2", target_bir_lowering=False, debug=False,
                   num_devices=NC)

    xq_in = nc.dram_tensor("xq", [D, SB], BF16, kind="ExternalInput").ap()
    wq_in = nc.dram_tensor("wqT", [D, D], BF16, kind="ExternalInput").ap()
    wv_in = nc.dram_tensor("wvT", [D, D], BF16, kind="ExternalInput").ap()
    wk_in = nc.dram_tensor("wkT", [D, D], BF16, kind="ExternalInput").ap()
    mask_in = nc.dram_tensor("drop_mask", [4 * 128, S], BF16,
                             kind="ExternalInput").ap()
    sched_in = nc.dram_tensor("sched", [128, NPAIR], F32,
                              kind="ExternalInput").ap()
    out_ext = nc.dram_tensor("out", [4 * 128, D], BF16,
                             kind="ExternalOutput").ap()

    with tile.TileContext(nc) as tc:
        with ExitStack() as es:
            dram = es.enter_context(tc.tile_pool(name="dram", bufs=1,
                                                 space="DRAM"))
            const = es.enter_context(tc.tile_pool(name="const", bufs=1))
            psum = es.enter_context(tc.tile_pool(name="psum", bufs=1,
                                                 space="PSUM"))

            # ---------------- DRAM scratch ----------------
            dummy_in = dram.tile([1, NPAIR], F32, name="dummy_in")
            dummy_out = dram.tile([NC, NPAIR], F32, addr_space="Shared",
                                  name="dummy_out")
            # V contributions split by slot-pair (g=0: slots {0,1} = tiles
            # 0..15; g=1: slots {2,3}) AND by d_out half (h), giving four
            # 4MB AllGathers; vtA/vtB tiles map 1:1 onto the h-chunks.
            vq_in = [[dram.tile([256, BK], BF16, name=f"vq_in{g}_{h}")
                      for h in range(2)] for g in range(2)]
            vgh = [[dram.tile([NC * 256, BK], BF16, addr_space="Shared",
                              name=f"vg{g}_{h}") for h in range(2)]
                   for g in range(2)]
            # per-core K^T contributions at FULL d_out depth: group 0
            # (slots {0,1}) split into d_out halves (two 4MB AGs fired
            # during the K projection), group 1 as ONE 8MB AG whose input
            # is deliberately written only after V slot 0 — the CC core
            # runs whichever pending op triggered earliest, and ops whose
            # triggers tie race differently across cores (device crash),
            # so every AG gets a distinct compute-gated ready time.
            kq = [[dram.tile([BK, 256], BF16, name=f"kq{g}_{H}")
                   for H in range(2)] for g in range(2)]
            kgx = [[dram.tile([NC * BK, 256], BF16, addr_space="Shared",
                              name=f"kg{g}_{H}") for H in range(2)]
                   for g in range(2)]

            # dummy AllGather first, with NO input dependency (dummy_in is
            # never written): the CC trigger fires at t~0 so the
            # collectives-init barrier overlaps the K projection.
            nc.gpsimd.collective_compute(
                "AllGather", ALU.bypass, replica_groups=RG,
                ins=[dummy_in.opt()], outs=[dummy_out.opt()],
            )

            # ---------------- weight / activation loads ----------------
            sched_sb = const.tile([128, NPAIR], F32, name="sched_sb")
            nc.sync.dma_start(sched_sb[:], sched_in)

            att = es.enter_context(tc.tile_pool(name="att", bufs=1))
            qt_sb = att.tile([128, 16, SB], BF16, name="qt_sb")

            # Shared ring: the four Wv quarters ([128, 8, 1024] each, by
            # ki-half x d_out-half) and the attention kt tiles have the
            # SAME shape, so kt tiles rotate directly into the wv buffers
            # the moment the V projection stops reading them — kt loads
            # start ~70us before the Q projection finishes, costing zero
            # extra SBUF.  Lives until the end of the kernel.
            wvkt = es.enter_context(tc.tile_pool(name="wvkt", bufs=4))
            wvq = [[wvkt.tile([128, 8, BK], BF16, tag="wvkt",
                              name=f"wv{ih}_{h}") for h in range(2)]
                   for ih in range(2)]

            qes = ExitStack()
            xqp = qes.enter_context(tc.tile_pool(name="xqp", bufs=1))
            xq_sb = xqp.tile([128, 16, SB], BF16, name="xq_sb")
            stagep = qes.enter_context(tc.tile_pool(name="stagep", bufs=2))
            # wk pool: wqA/wqB rotate into wkA/wkB's buffers once the K
            # projection finishes.
            wkq = qes.enter_context(tc.tile_pool(name="wkq", bufs=2))

            wkA = wkq.tile([128, 8, D], BF16, tag="wh", name="wkA")
            wkB = wkq.tile([128, 8, D], BF16, tag="wh", name="wkB")

            # fine-grained interleaved startup loads across all 3 queues:
            # sync: ki 0..4, gpsimd: ki 5..9, scalar: ki 10..15, then
            # scalar continues with the wv quarters (wq + masks later).
            def wkx_chunk(eng, ki):
                wdst = wkA if ki < 8 else wkB
                eng.dma_start(wdst[:, ki % 8, :],
                              wk_in[128 * ki:128 * (ki + 1), :])
                eng.dma_start(xq_sb[:, ki, :],
                              xq_in[128 * ki:128 * (ki + 1), :])

            for ki in range(5):
                wkx_chunk(nc.sync, ki)
            for ki in range(5, 10):
                wkx_chunk(nc.gpsimd, ki)
            for ki in range(10, 16):
                wkx_chunk(nc.scalar, ki)
            for ih in range(2):
                for h in range(2):
                    nc.scalar.dma_start(
                        wvq[ih][h][:],
                        wv_in[1024 * ih:1024 * (ih + 1),
                              1024 * h:1024 * (h + 1)]
                        .rearrange("(k p) d -> p k d", p=128))

            # ---------------- constants (gpsimd engine, after triggers) ---
            ident_sb = const.tile([128, 128], BF16, name="ident_sb")
            make_identity(nc, ident_sb[:])
            iota_sb = const.tile([128, BK], F32, name="iota_sb")
            nc.gpsimd.iota(
                iota_sb[:], pattern=[[-1, BK]], base=0, channel_multiplier=1,
                allow_small_or_imprecise_dtypes=True,
            )

            partials = const.tile([128, NPAIR], F32, name="partials")
            den = const.tile([128, 4], F32, name="den")
            rec = const.tile([128, 4], F32, name="rec")

            # ------- phase K: local K^T (all d_out, own q) -------
            # kst cols 0:256 -> kq0 (written immediately, AGs fire at dp3
            # and dp7); cols 256:512 are parked in SBUF (k1all) and only
            # written to kq1 after V slot 0, spacing KAG1's trigger well
            # after KAG0b's and well before VAG0a's.
            k1all = stagep.tile([128, 16, 256], BF16, tag="k1all", bufs=1,
                                name="k1all")
            for dp in range(8):
                psA = psum.tile([128, BK], F32, tag="pw", bufs=3,
                                name=f"pskA{dp}")
                psB = psum.tile([128, BK], F32, tag="pw", bufs=3,
                                name=f"pskB{dp}")
                for i, ki in enumerate(KI_ORDER):
                    wkh = wkA if ki < 8 else wkB
                    nc.tensor.matmul(
                        psA[:, 0:SB], lhsT=wkh[:, ki % 8, 256 * dp:
                                               256 * dp + 128],
                        rhs=xq_sb[:, ki, :],
                        start=(i == 0), stop=(i == 15),
                    )
                    nc.tensor.matmul(
                        psB[:, 0:SB], lhsT=wkh[:, ki % 8, 256 * dp + 128:
                                               256 * dp + 256],
                        rhs=xq_sb[:, ki, :],
                        start=(i == 0), stop=(i == 15),
                    )
                for half, ps in ((0, psA), (1, psB)):
                    dt = 2 * dp + half
                    kst = stagep.tile([128, 256], BF16, tag="kst", bufs=6,
                                      name=f"kst{dt}")
                    nc.vector.tensor_copy(kst[:], ps[:, 0:256])
                    nc.vector.tensor_copy(k1all[:, dt, :], ps[:, 256:512])
                    nc.sync.dma_start(
                        kq[0][dt // 8][128 * (dt % 8):
                                       128 * (dt % 8) + 128, :],
                        kst[:])
                if dp in (3, 7):
                    H = dp // 4
                    nc.gpsimd.collective_compute(
                        "AllGather", ALU.bypass, replica_groups=RG,
                        ins=[kq[0][H].opt()], outs=[kgx[0][H].opt()],
                    )

            def kq1_write(H):
                for dt in range(8 * H, 8 * H + 8):
                    nc.sync.dma_start(
                        kq[1][H][128 * (dt % 8):128 * (dt % 8) + 128, :],
                        k1all[:, dt, :])
                nc.gpsimd.collective_compute(
                    "AllGather", ALU.bypass, replica_groups=RG,
                    ins=[kq[1][H].opt()], outs=[kgx[1][H].opt()],
                )

            # ------- phase V: natural layout, slots 0..3 ------
            # h=1 vq writes are DEFERRED two phases to ladder the V-gather
            # triggers ~17us apart (see kq/kg comment).
            def vq_write(st, h, vst):
                nc.sync.dma_start(
                    vq_in[st // 2][h]
                    .rearrange("(t p) d -> p t d", p=128)[:, st % 2, :],
                    vst[:])

            def v_slot(st, defer):
                deferred = []
                for h in range(2):
                    ps = psum.tile([128, BK], F32, tag="pw", bufs=3,
                                   name=f"psv{st}_{h}")
                    for ki in range(16):
                        for n2 in range(2):
                            nc.tensor.matmul(
                                ps[:, 512 * n2:512 * (n2 + 1)],
                                lhsT=xq_sb[:, ki, 128 * st:128 * (st + 1)],
                                rhs=wvq[ki // 8][h][
                                    :, ki % 8, 512 * n2:512 * (n2 + 1)],
                                start=(ki == 0), stop=(ki == 15),
                                skip_group_check=True,
                            )
                    vst = stagep.tile([128, BK], BF16, tag="vst", bufs=6,
                                      name=f"vst{st}_{h}")
                    nc.vector.tensor_copy(vst[:], ps[:])
                    if h in defer:
                        deferred.append(vst)
                    else:
                        vq_write(st, h, vst)
                return deferred

            # Chain order = consumption order; each op's input completes at
            # a distinct compute-gated point ~15us after the previous one:
            #   KAG0a (K dp3), KAG0b (K dp7), VAG0a (Vc1 st1-h0),
            #   VAG0b (after st2), KAG1a (after st3), KAG1b (Q dp1),
            #   VAG1a (Q dp3), VAG1b (Q dp5).
            vd0 = v_slot(0, defer={1})          # st0: h1 deferred
            vd1 = v_slot(1, defer={1})          # st1: h1 deferred
            nc.gpsimd.collective_compute(
                "AllGather", ALU.bypass, replica_groups=RG,
                ins=[vq_in[0][0].opt()], outs=[vgh[0][0].opt()],
            )
            vd2 = v_slot(2, defer={0, 1})       # st2: both deferred
            vq_write(0, 1, vd0[0])
            vq_write(1, 1, vd1[0])
            nc.gpsimd.collective_compute(
                "AllGather", ALU.bypass, replica_groups=RG,
                ins=[vq_in[0][1].opt()], outs=[vgh[0][1].opt()],
            )
            vd3 = v_slot(3, defer={0, 1})       # st3: both deferred
            kq1_write(0)

            # kt tiles: ktA = d_out rows 0..1023 (score ki 0..7), ktB =
            # rows 1024..2047, from kg0[H] (blocks 0/1) or kg1 (blocks 2/3).
            # All 8 tiles of a block share slot parity: one 128-col stripe.
            def emit_kt(B):
                ktA = wvkt.tile([128, 8, BK], BF16, tag="wvkt",
                                name=f"ktA{B}")
                ktB = wvkt.tile([128, 8, BK], BF16, tag="wvkt",
                                name=f"ktB{B}")
                g = B // 2
                coff = 128 * (B % 2)
                # ktB goes on scalar: its trigger fires at ring-release,
                # always BEFORE the pex activations behind it in the FIFO
                # reach the head, so no head-of-line risk (unlike vt loads,
                # whose AG waits would block pex — those stay off scalar).
                for H, kth, eng in ((0, ktA, nc.sync), (1, ktB, nc.scalar)):
                    for j in range(8):
                        t = 8 * B + j
                        c, _s = tile_owner_slot(t)
                        eng.dma_start(
                            kth[:, :, 128 * j:128 * (j + 1)],
                            kgx[g][H][BK * c:BK * (c + 1), coff:coff + 128]
                            .rearrange("(k p) q -> p k q", p=128),
                        )
                return ktA, ktB

            # blocks 0/1 kt tiles rotate into the wv buffers right here:
            # their loads run during the Q projection.
            kt0 = emit_kt(0)
            kt1 = emit_kt(1)

            # Wq^T halves rotate into Wk^T's buffers (dep: K matmuls done).
            wqA = wkq.tile([128, 8, D], BF16, tag="wh", name="wqA")
            wqB = wkq.tile([128, 8, D], BF16, tag="wh", name="wqB")
            nc.scalar.dma_start(
                wqA[:], wq_in[0:1024, :].rearrange("(k p) d -> p k d", p=128))
            nc.scalar.dma_start(
                wqB[:], wq_in[1024:2048, :].rearrange("(k p) d -> p k d",
                                                      p=128))

            # ------- phase Q: local Q^T projection ------
            for dp in range(8):
                psA = psum.tile([128, BK], F32, tag="pw", bufs=3,
                                name=f"psqA{dp}")
                psB = psum.tile([128, BK], F32, tag="pw", bufs=3,
                                name=f"psqB{dp}")
                for ki in range(16):
                    wqh = wqA if ki < 8 else wqB
                    nc.tensor.matmul(
                        psA[:, 0:SB], lhsT=wqh[:, ki % 8, 256 * dp:
                                               256 * dp + 128],
                        rhs=xq_sb[:, ki, :],
                        start=(ki == 0), stop=(ki == 15),
                    )
                    nc.tensor.matmul(
                        psB[:, 0:SB], lhsT=wqh[:, ki % 8, 256 * dp + 128:
                                               256 * dp + 256],
                        rhs=xq_sb[:, ki, :],
                        start=(ki == 0), stop=(ki == 15),
                    )
                nc.vector.tensor_copy(qt_sb[:, 2 * dp, :], psA[:, 0:SB])
                nc.vector.tensor_copy(qt_sb[:, 2 * dp + 1, :],
                                      psB[:, 0:SB])
                if dp == 1:
                    kq1_write(1)
                elif dp == 3:
                    vq_write(2, 0, vd2[0])
                    vq_write(3, 0, vd3[0])
                    nc.gpsimd.collective_compute(
                        "AllGather", ALU.bypass, replica_groups=RG,
                        ins=[vq_in[1][0].opt()], outs=[vgh[1][0].opt()],
                    )
                elif dp == 5:
                    vq_write(2, 1, vd2[1])
                    vq_write(3, 1, vd3[1])
                    nc.gpsimd.collective_compute(
                        "AllGather", ALU.bypass, replica_groups=RG,
                        ins=[vq_in[1][1].opt()], outs=[vgh[1][1].opt()],
                    )
            qes.close()

            # ---------------- attention (software-pipelined) ----------------
            accp = es.enter_context(tc.tile_pool(name="accp", bufs=1))
            vtl = es.enter_context(tc.tile_pool(name="vtl", bufs=4))
            mkl = es.enter_context(tc.tile_pool(name="mkl", bufs=3))
            pwork = es.enter_context(tc.tile_pool(name="pwork", bufs=2))

            acc = [accp.tile([128, D], F32, name=f"acc{t}") for t in range(4)]

            # Dropout masks load lazily: 3 upfront, then pair i+3's mask is
            # emitted right after pair i's pm frees its ring slot, so the
            # scalar-queue trigger NEVER waits (a waiting mask trigger
            # head-of-line-blocks the pex activations: measured 30us stall).
            PAIRLIST = [(B, s) for B in range(NBIG) for s in range(B, 4)]
            mk_all = {}

            def emit_mask(i):
                if i >= len(PAIRLIST):
                    return
                B, slot = PAIRLIST[i]
                mk = mkl.tile([128, BK], BF16, tag="mk",
                              name=f"mk{B}_{slot}")
                nc.scalar.dma_start(
                    mk[:],
                    mask_in[128 * slot:128 * (slot + 1),
                            BK * B:BK * (B + 1)],
                )
                mk_all[(B, slot)] = mk

            for i in range(3):
                emit_mask(i)

            def emit_vt_half(B, h, eng):
                vt = vtl.tile([128, 8, BK], BF16, tag="vt",
                              name=f"vt{'AB'[h]}{B}")
                for j in range(8):
                    r0 = VROW2[8 * B + j]
                    eng.dma_start(vt[:, j, :], vgh[B // 2][h][r0:r0 + 128, :])
                return vt

            def normalize_slot(slot):
                obf = pwork.tile([128, D], BF16, tag="obf", bufs=1,
                                 name=f"obf{slot}")
                nc.vector.tensor_reduce(
                    den[:, slot:slot + 1],
                    partials[:, PBASE[slot]:PBASE[slot] + KBMAX[slot]],
                    axis=mybir.AxisListType.X, op=ALU.add,
                )
                nc.vector.reciprocal(rec[:, slot:slot + 1],
                                     den[:, slot:slot + 1])
                nc.vector.tensor_scalar_mul(
                    obf[:], acc[slot][:], rec[:, slot:slot + 1])
                nc.scalar.dma_start(
                    out_ext[128 * slot:128 * (slot + 1), :], obf[:])

            def tp_stage(st):
                pm, vtA, vtB, B, slot = st
                pmt = pwork.tile([128, 8, 128], BF16, tag="pmt", bufs=3,
                                 name=f"pmt{B}_{slot}")
                for j in range(8):
                    tp = psum.tile([128, 128], BF16, tag="tp", bufs=2,
                                   name=f"tp{B}_{slot}_{j}")
                    nc.tensor.matmul(
                        tp[:], lhsT=pm[:, 128 * j:128 * (j + 1)],
                        rhs=ident_sb[:], is_transpose=True,
                        skip_group_check=True)
                    nc.scalar.copy(pmt[:, j, :], tp[:])
                return pmt

            def av_stage(st, pmt):
                pm, vtA, vtB, B, slot = st
                for h, vt in ((0, vtA), (1, vtB)):
                    av = psum.tile([128, BK], F32, tag="pw", bufs=3,
                                   name=f"av{B}_{slot}_{h}")
                    for j in range(8):
                        for n2 in range(2):
                            nc.tensor.matmul(
                                av[:, 512 * n2:512 * (n2 + 1)],
                                lhsT=pmt[:, j, :],
                                rhs=vt[:, j, 512 * n2:512 * (n2 + 1)],
                                start=(j == 0), stop=(j == 7),
                                skip_group_check=True,
                            )
                    if B == 0:
                        nc.vector.tensor_copy(
                            acc[slot][:, BK * h:BK * (h + 1)], av[:])
                    else:
                        nc.vector.scalar_tensor_tensor(
                            out=acc[slot][:, BK * h:BK * (h + 1)],
                            in0=av[:], scalar=1.0,
                            in1=acc[slot][:, BK * h:BK * (h + 1)],
                            op0=ALU.mult, op1=ALU.add,
                        )

            # software pipeline state: pair p's P-transposes run during
            # pair p+1's scores; its attn@V runs after pair p+2's scores
            # (the 2-pair lag lets the vt loads finish behind the Q-end
            # SBUF release without stalling the PE).
            state = {"prev": None, "prev_pmt": None, "old": None,
                     "old_pmt": None}

            def retire_old():
                if state["old"] is not None:
                    av_stage(state["old"], state["old_pmt"])
                    oB, oslot = state["old"][3], state["old"][4]
                    if oB == oslot:
                        # slot oslot's accumulation is complete (its
                        # diagonal block was its last): normalize and
                        # write it out now, hidden under later pairs.
                        normalize_slot(oslot)

            def emit_pairs(B, ktA, ktB, vtA, vtB):
                for slot in range(B, 4):
                    p = PBASE[slot] + B
                    mk = mk_all[(B, slot)]
                    sc = psum.tile([128, BK], F32, tag="pw", bufs=3,
                                   name=f"sc{B}_{slot}")
                    for ki in range(16):
                        if ki == 8 and state["prev"] is not None:
                            # interleave prev pair's P-transposes here so
                            # the pmt copies finish before its attn@V
                            state["prev_pmt"] = tp_stage(state["prev"])
                        kth = ktA if ki < 8 else ktB
                        for n2 in range(2):
                            nc.tensor.matmul(
                                sc[:, 512 * n2:512 * (n2 + 1)],
                                lhsT=qt_sb[:, ki, 128 * slot:128 * (slot + 1)],
                                rhs=kth[:, ki % 8, 512 * n2:512 * (n2 + 1)],
                                start=(ki == 0), stop=(ki == 15),
                                skip_group_check=True,
                            )
                    pex = pwork.tile([128, BK], BF16, tag="pex", bufs=1,
                                     name=f"pex{B}_{slot}")
                    nc.scalar.activation(pex[:], sc[:], AFT.Exp, scale=SCALE)
                    pcs = pwork.tile([128, BK], BF16, tag="pcs", bufs=1,
                                     name=f"pcs{B}_{slot}")
                    nc.vector.scalar_tensor_tensor(
                        out=pcs[:], in0=iota_sb[:],
                        scalar=sched_sb[:, p:p + 1], in1=pex[:],
                        op0=ALU.is_ge, op1=ALU.mult,
                        accum_out=partials[:, p:p + 1],
                    )
                    pm = pwork.tile([128, BK], BF16, tag="pm", bufs=3,
                                    name=f"pm{B}_{slot}")
                    nc.vector.tensor_mul(pm[:], pcs[:], mk[:])
                    emit_mask(PAIRLIST.index((B, slot)) + 3)
                    retire_old()
                    state["old"] = state["prev"]
                    state["old_pmt"] = state["prev_pmt"]
                    state["prev"] = (pm, vtA, vtB, B, slot)

            # group 1: blocks 0 and 1 (kt tiles already loading since the
            # V phase ended; vt tiles load from attention start).
            vtA0 = emit_vt_half(0, 0, nc.gpsimd)
            vtB0 = emit_vt_half(0, 1, nc.gpsimd)
            vtA1 = emit_vt_half(1, 0, nc.sync)
            vtB1 = emit_vt_half(1, 1, nc.gpsimd)

            emit_pairs(0, kt0[0], kt0[1], vtA0, vtB0)
            emit_pairs(1, kt1[0], kt1[1], vtA1, vtB1)

            # group 2: blocks 2 and 3.  vt loads go on gpsimd — safe here
            # because no CC trigger is emitted after them.
            kt2 = emit_kt(2)
            kt3 = emit_kt(3)
            vtA2 = emit_vt_half(2, 0, nc.sync)
            vtA3 = emit_vt_half(3, 0, nc.sync)
            vtB2 = emit_vt_half(2, 1, nc.gpsimd)
            vtB3 = emit_vt_half(3, 1, nc.gpsimd)
            emit_pairs(2, kt2[0], kt2[1], vtA2, vtB2)
            emit_pairs(3, kt3[0], kt3[1], vtA3, vtB3)

            retire_old()
            state["old"] = state["prev"]
            state["old_pmt"] = tp_stage(state["prev"])
            retire_old()   # retires (3,3), which also normalizes slot 3

    nc.compile()
    return nc


_NC_CACHE = None


def _get_nc():
    global _NC_CACHE
    if _NC_CACHE is None:
        _NC_CACHE = build()
    return _NC_CACHE


def make_in_maps(x, Wq, Wk, Wv, drop_mask):
    bf = ml_dtypes.bfloat16
    x = np.asarray(x, dtype=np.float32)
    Wq = np.asarray(Wq, dtype=np.float32)
    Wk = np.asarray(Wk, dtype=np.float32)
    Wv = np.asarray(Wv, dtype=np.float32)
    drop_mask = np.asarray(drop_mask, dtype=np.float32)

    xT = np.ascontiguousarray(x.T).astype(bf)           # [D, S]
    wqT = np.ascontiguousarray(Wq.T.astype(bf))         # [D, D]
    wvT = np.ascontiguousarray(Wv.T.astype(bf))         # [D, D]
    wkT = np.ascontiguousarray(Wk.T.astype(bf))         # [D, D]
    mask_bf = drop_mask.astype(bf)

    in_maps = []
    for c in range(NC):
        tl = owned_tiles(c)
        thr = np.array(
            [1024.0 * B - 128.0 * tl[slot]
             for slot in range(4) for B in range(KBMAX[slot])],
            dtype=np.float32,
        )
        in_maps.append({
            "xq": np.ascontiguousarray(
                np.concatenate([xT[:, 128 * t:128 * (t + 1)] for t in tl],
                               axis=1)),
            "wqT": wqT,
            "wvT": wvT,
            "wkT": wkT,
            "drop_mask": np.ascontiguousarray(
                np.concatenate(
                    [mask_bf[128 * t:128 * (t + 1)] for t in tl], axis=0)),
            "sched": np.ascontiguousarray(np.tile(thr[None, :], (128, 1))),
        })
    return in_maps


def assemble(results):
    full = np.zeros((S, D), dtype=np.float32)
    for c in range(NC):
        o = np.asarray(results[c]["out"], dtype=np.float32)
        for slot, t in enumerate(owned_tiles(c)):
            full[128 * t:128 * (t + 1)] = o[128 * slot:128 * (slot + 1)]
    return full


def kernel(x, Wq, Wk, Wv, drop_mask):
    nc = _get_nc()
    in_maps = make_in_maps(x, Wq, Wk, Wv, drop_mask)
    res = bass_utils.run_bass_kernel_spmd(nc, in_maps, core_ids=list(range(NC)))
    return assemble(res.results)


def kernel_profiled(x, Wq, Wk, Wv, drop_mask):
    """Like kernel(), but captures an NTFF profile; returns (out, exec_time_ns,
    trace_path)."""
    nc = _get_nc()
    in_maps = make_in_maps(x, Wq, Wk, Wv, drop_mask)
    res = bass_utils.run_bass_kernel_spmd(
        nc, in_maps, core_ids=list(range(NC)), trace=True)
    trace_path = None
    if res.instructions_and_trace is not None:
        trace_path = res.instructions_and_trace[1]
    return assemble(res.results), res.exec_time_ns, trace_path


# revision 80
# speedup vs baseline: 1.0444x; 1.0444x over previous
"""Distributed causal-attention-with-dropout kernel for 8 TRN2 NeuronCores, v19.

Architecture ("all-local projections", fully static SPMD graph):

- Host pre-formats inputs (layout only, all model FLOPs stay on device):
  each core receives xq = x^T columns of its 4 OWNED q-tiles
  {c, 15-c, 16+c, 31-c} (bf16), the FULL Wq^T / Wk^T / Wv^T (bf16,
  replicated), dropout-mask rows of its owned tiles (bf16), and the causal
  threshold table.  There is NO x gather at all.
- A tiny dummy AllGather with NO input dependency fires at t~0 so the
  collectives-runtime init barrier (~20+40us) overlaps the K projection.
- Tensor phase order: K proj -> V slots {0,1} -> V slots {2,3} -> Q proj
  -> attention.  Startup loads are interleaved (wk ki-chunk, xq ki-chunk)
  across all 3 DMA queues and the K projection consumes ki in arrival
  order, so the PE starts at ~4us and stays dense (HAM un-throttled).
- K^T is AllGathered at FULL d_out depth in k-tile-group chunks, with
  group 0 further split into d_out halves so the first AG fires mid-K-proj:
  CC chain = KAG0a (4MB, after dp3), KAG0b (4MB), VAG0, KAG1 (8MB),
  VAG1 — exactly the attention consumption order, with no CC idle.
- Score big-block B needs only kg group B//2; block tiles share slot
  parity so each block reads one 128-column stripe of the AG output.
- kt tiles for blocks 0/1 load DURING the Q projection (the wv weight
  pool closes after V, freeing SBUF for ktl).  The KAG1/VAG1 triggers are
  EMITTED after the B=0/1 loads+pairs: DRAM DMAs conservatively wait on
  in-flight collectives, and CC-completion-dependent DMAs must NEVER sit
  on the gpsimd queue (they'd block the later CC triggers behind them).
- Attention: k-blocks are 1024 wide -> ZERO-padding static schedule
  (slot s needs exactly s+1 blocks; 10 pairs).  Causality enforced
  per-row by (iota(p-j) >= thr) * P on the vector engine; softmax without
  max-subtraction; denominators use pre-dropout sums.  Pair p's
  P-transposes and attn@V run after pair p+1's score matmuls.
"""

import math
import os
import sys
from contextlib import ExitStack

import numpy as np
import ml_dtypes

for _p in ("/opt/trn_rl_repo", "/root/.axon_site/_ro/trn_rl_repo"):
    if os.path.isdir(_p) and _p not in sys.path:
        sys.path.append(_p)

import concourse.bass as bass
import concourse.tile as tile
from concourse import bacc, mybir
from concourse import bass_utils
from concourse.masks import make_identity

S, D = 4096, 2048
NC = 8
SB = 512          # seq rows per core (4 owned 128-tiles)
BK = 1024         # big k-block width
NBIG = 4
KBMAX = (1, 2, 3, 4)
PBASE = (0, 1, 3, 6)
NPAIR = 10
SCALE = 1.0 / math.sqrt(float(D))
F32 = mybir.dt.float32
BF16 = mybir.dt.bfloat16
RG = [list(range(NC))]
ALU = mybir.AluOpType
AFT = mybir.ActivationFunctionType

# ki consumption order for the K projection: matches the 3-queue load
# arrival pattern (sync: ki 0..4, gpsimd: ki 5..9, scalar: ki 10..15) so
# the first dp group issues matmuls as chunks land.
KI_ORDER = (0, 5, 10, 1, 6, 11, 2, 7, 12, 3, 8, 13, 4, 9, 14, 15)


def owned_tiles(c):
    return (c, 15 - c, 16 + c, 31 - c)


def tile_owner_slot(t):
    if t <= 7:
        return t, 0
    if t <= 15:
        return 15 - t, 1
    if t <= 23:
        return t - 16, 2
    return 31 - t, 3


# row of tile t inside its V AllGather chunk (chunk = t//16; within a
# chunk, rank blocks of 256 rows hold slots {0,1} or {2,3})
VROW2 = [256 * tile_owner_slot(t)[0] + 128 * (tile_owner_slot(t)[1] % 2)
         for t in range(32)]


def build():
    nc = bacc.Bacc("TRN2", target_bir_lowering=False, debug=False,
                   num_devices=NC)

    xq_in = nc.dram_tensor("xq", [D, SB], BF16, kind="ExternalInput").ap()
    wq_in = nc.dram_tensor("wqT", [D, D], BF16, kind="ExternalInput").ap()
    wv_in = nc.dram_tensor("wvT", [D, D], BF16, kind="ExternalInput").ap()
    wk_in = nc.dram_tensor("wkT", [D, D], BF16, kind="ExternalInput").ap()
    mask_in = nc.dram_tensor("drop_mask", [4 * 128, S], BF16,
                             kind="ExternalInput").ap()
    sched_in = nc.dram_tensor("sched", [128, NPAIR], F32,
                              kind="ExternalInput").ap()
    out_ext = nc.dram_tensor("out", [4 * 128, D], BF16,
                             kind="ExternalOutput").ap()

    with tile.TileContext(nc) as tc:
        with ExitStack() as es:
            dram = es.enter_context(tc.tile_pool(name="dram", bufs=1,
                                                 space="DRAM"))
            const = es.enter_context(tc.tile_pool(name="const", bufs=1))
            psum = es.enter_context(tc.tile_pool(name="psum", bufs=1,
                                                 space="PSUM"))

            # ---------------- DRAM scratch ----------------
            dummy_in = dram.tile([1, NPAIR], F32, name="dummy_in")
            dummy_out = dram.tile([NC, NPAIR], F32, addr_space="Shared",
                                  name="dummy_out")
            # V contributions split by slot-pair (g=0: slots {0,1} = tiles
            # 0..15; g=1: slots {2,3}) AND by d_out half (h), giving four
            # 4MB AllGathers; vtA/vtB tiles map 1:1 onto the h-chunks.
            vq_in = [[dram.tile([256, BK], BF16, name=f"vq_in{g}_{h}")
                      for h in range(2)] for g in range(2)]
            vgh = [[dram.tile([NC * 256, BK], BF16, addr_space="Shared",
                              name=f"vg{g}_{h}") for h in range(2)]
                   for g in range(2)]
            # per-core K^T contributions at FULL d_out depth: group 0
            # (slots {0,1}) split into d_out halves (two 4MB AGs fired
            # during the K projection), group 1 as ONE 8MB AG whose input
            # is deliberately written only after V slot 0 — the CC core
            # runs whichever pending op triggered earliest, and ops whose
            # triggers tie race differently across cores (device crash),
            # so every AG gets a distinct compute-gated ready time.
            kq = [[dram.tile([BK, 256], BF16, name=f"kq{g}_{H}")
                   for H in range(2)] for g in range(2)]
            kgx = [[dram.tile([NC * BK, 256], BF16, addr_space="Shared",
                              name=f"kg{g}_{H}") for H in range(2)]
                   for g in range(2)]

            # dummy AllGather first, with NO input dependency (dummy_in is
            # never written): the CC trigger fires at t~0 so the
            # collectives-init barrier overlaps the K projection.
            nc.gpsimd.collective_compute(
                "AllGather", ALU.bypass, replica_groups=RG,
                ins=[dummy_in.opt()], outs=[dummy_out.opt()],
            )

            # ---------------- weight / activation loads ----------------
            sched_sb = const.tile([128, NPAIR], F32, name="sched_sb")
            nc.sync.dma_start(sched_sb[:], sched_in)

            att = es.enter_context(tc.tile_pool(name="att", bufs=1))
            qt_sb = att.tile([128, 16, SB], BF16, name="qt_sb")

            # Shared ring: the four Wv quarters ([128, 8, 1024] each, by
            # ki-half x d_out-half) and the attention kt tiles have the
            # SAME shape, so kt tiles rotate directly into the wv buffers
            # the moment the V projection stops reading them — kt loads
            # start ~70us before the Q projection finishes, costing zero
            # extra SBUF.  Lives until the end of the kernel.
            wvkt = es.enter_context(tc.tile_pool(name="wvkt", bufs=4))
            wvq = [[wvkt.tile([128, 8, BK], BF16, tag="wvkt",
                              name=f"wv{ih}_{h}") for h in range(2)]
                   for ih in range(2)]

            qes = ExitStack()
            xqp = qes.enter_context(tc.tile_pool(name="xqp", bufs=1))
            xq_sb = xqp.tile([128, 16, SB], BF16, name="xq_sb")
            stagep = qes.enter_context(tc.tile_pool(name="stagep", bufs=2))
            # wk pool: wqA/wqB rotate into wkA/wkB's buffers once the K
            # projection finishes.
            wkq = qes.enter_context(tc.tile_pool(name="wkq", bufs=2))

            wkA = wkq.tile([128, 8, D], BF16, tag="wh", name="wkA")
            wkB = wkq.tile([128, 8, D], BF16, tag="wh", name="wkB")

            # fine-grained interleaved startup loads across all 3 queues:
            # sync: ki 0..4, gpsimd: ki 5..9, scalar: ki 10..15, then
            # scalar continues with the wv quarters (wq + masks later).
            def wkx_chunk(eng, ki):
                wdst = wkA if ki < 8 else wkB
                eng.dma_start(wdst[:, ki % 8, :],
                              wk_in[128 * ki:128 * (ki + 1), :])
                eng.dma_start(xq_sb[:, ki, :],
                              xq_in[128 * ki:128 * (ki + 1), :])

            for ki in range(5):
                wkx_chunk(nc.sync, ki)
            for ki in range(5, 10):
                wkx_chunk(nc.gpsimd, ki)
            for ki in range(10, 16):
                wkx_chunk(nc.scalar, ki)
            for ih in range(2):
                for h in range(2):
                    nc.scalar.dma_start(
                        wvq[ih][h][:],
                        wv_in[1024 * ih:1024 * (ih + 1),
                              1024 * h:1024 * (h + 1)]
                        .rearrange("(k p) d -> p k d", p=128))

            # ---------------- constants (gpsimd engine, after triggers) ---
            ident_sb = const.tile([128, 128], BF16, name="ident_sb")
            make_identity(nc, ident_sb[:])
            iota_sb = const.tile([128, BK], F32, name="iota_sb")
            nc.gpsimd.iota(
                iota_sb[:], pattern=[[-1, BK]], base=0, channel_multiplier=1,
                allow_small_or_imprecise_dtypes=True,
            )

            partials = const.tile([128, NPAIR], F32, name="partials")
            den = const.tile([128, 4], F32, name="den")
            rec = const.tile([128, 4], F32, name="rec")

            # ------- phase K: local K^T (all d_out, own q) -------
            # kst cols 0:256 -> kq0 (written immediately, AGs fire at dp3
            # and dp7); cols 256:512 are parked in SBUF (k1all) and only
            # written to kq1 after V slot 0, spacing KAG1's trigger well
            # after KAG0b's and well before VAG0a's.
            k1all = stagep.tile([128, 16, 256], BF16, tag="k1all", bufs=1,
                                name="k1all")
            for dp in range(8):
                psA = psum.tile([128, BK], F32, tag="pw", bufs=3,
                                name=f"pskA{dp}")
                psB = psum.tile([128, BK], F32, tag="pw", bufs=3,
                                name=f"pskB{dp}")
                for i, ki in enumerate(KI_ORDER):
                    wkh = wkA if ki < 8 else wkB
                    nc.tensor.matmul(
                        psA[:, 0:SB], lhsT=wkh[:, ki % 8, 256 * dp:
                                               256 * dp + 128],
                        rhs=xq_sb[:, ki, :],
                        start=(i == 0), stop=(i == 15),
                    )
                    nc.tensor.matmul(
                        psB[:, 0:SB], lhsT=wkh[:, ki % 8, 256 * dp + 128:
                                               256 * dp + 256],
                        rhs=xq_sb[:, ki, :],
                        start=(i == 0), stop=(i == 15),
                    )
                for half, ps in ((0, psA), (1, psB)):
                    dt = 2 * dp + half
                    kst = stagep.tile([128, 256], BF16, tag="kst", bufs=6,
                                      name=f"kst{dt}")
                    nc.vector.tensor_copy(kst[:], ps[:, 0:256])
                    nc.vector.tensor_copy(k1all[:, dt, :], ps[:, 256:512])
                    nc.sync.dma_start(
                        kq[0][dt // 8][128 * (dt % 8):
                                       128 * (dt % 8) + 128, :],
                        kst[:])
                if dp in (3, 7):
                    H = dp // 4
                    nc.gpsimd.collective_compute(
                        "AllGather", ALU.bypass, replica_groups=RG,
                        ins=[kq[0][H].opt()], outs=[kgx[0][H].opt()],
                    )

            def kq1_write(H):
                for dt in range(8 * H, 8 * H + 8):
                    nc.sync.dma_start(
                        kq[1][H][128 * (dt % 8):128 * (dt % 8) + 128, :],
                        k1all[:, dt, :])
                nc.gpsimd.collective_compute(
                    "AllGather", ALU.bypass, replica_groups=RG,
                    ins=[kq[1][H].opt()], outs=[kgx[1][H].opt()],
                )

            # ------- phase V: natural layout, slots 0..3 ------
            # h=1 vq writes are DEFERRED two phases to ladder the V-gather
            # triggers ~17us apart (see kq/kg comment).
            def vq_write(st, h, vst):
                nc.sync.dma_start(
                    vq_in[st // 2][h]
                    .rearrange("(t p) d -> p t d", p=128)[:, st % 2, :],
                    vst[:])

            def v_slot(st, defer):
                deferred = []
                for h in range(2):
                    ps = psum.tile([128, BK], F32, tag="pw", bufs=3,
                                   name=f"psv{st}_{h}")
                    for ki in range(16):
                        for n2 in range(2):
                            nc.tensor.matmul(
                                ps[:, 512 * n2:512 * (n2 + 1)],
                                lhsT=xq_sb[:, ki, 128 * st:128 * (st + 1)],
                                rhs=wvq[ki // 8][h][
                                    :, ki % 8, 512 * n2:512 * (n2 + 1)],
                                start=(ki == 0), stop=(ki == 15),
                                skip_group_check=True,
                            )
                    vst = stagep.tile([128, BK], BF16, tag="vst", bufs=6,
                                      name=f"vst{st}_{h}")
                    nc.vector.tensor_copy(vst[:], ps[:])
                    if h in defer:
                        deferred.append(vst)
                    else:
                        vq_write(st, h, vst)
                return deferred

            # Chain order = consumption order; each op's input completes at
            # a distinct compute-gated point ~15us after the previous one:
            #   KAG0a (K dp3), KAG0b (K dp7), VAG0a (Vc1 st1-h0),
            #   VAG0b (after st2), KAG1a (after st3), KAG1b (Q dp1),
            #   VAG1a (Q dp3), VAG1b (Q dp5).
            vd0 = v_slot(0, defer={1})          # st0: h1 deferred
            vd1 = v_slot(1, defer={1})          # st1: h1 deferred
            nc.gpsimd.collective_compute(
                "AllGather", ALU.bypass, replica_groups=RG,
                ins=[vq_in[0][0].opt()], outs=[vgh[0][0].opt()],
            )
            vd2 = v_slot(2, defer={0, 1})       # st2: both deferred
            vq_write(0, 1, vd0[0])
            vq_write(1, 1, vd1[0])
            nc.gpsimd.collective_compute(
                "AllGather", ALU.bypass, replica_groups=RG,
                ins=[vq_in[0][1].opt()], outs=[vgh[0][1].opt()],
            )
            vd3 = v_slot(3, defer={0, 1})       # st3: both deferred
            kq1_write(0)

            # kt tiles: ktA = d_out rows 0..1023 (score ki 0..7), ktB =
            # rows 1024..2047, from kg0[H] (blocks 0/1) or kg1 (blocks 2/3).
            # All 8 tiles of a block share slot parity: one 128-col stripe.
            def emit_kt(B):
                ktA = wvkt.tile([128, 8, BK], BF16, tag="wvkt",
                                name=f"ktA{B}")
                ktB = wvkt.tile([128, 8, BK], BF16, tag="wvkt",
                                name=f"ktB{B}")
                g = B // 2
                coff = 128 * (B % 2)
                # Attention loads go on sync+gpsimd ONLY: scalar-engine DMA
                # triggers would head-of-line-block the pex activations
                # behind their AG waits (measured 57us pipeline collapse).
                for H, kth, eng in ((0, ktA, nc.sync), (1, ktB, nc.gpsimd)):
                    for j in range(8):
                        t = 8 * B + j
                        c, _s = tile_owner_slot(t)
                        eng.dma_start(
                            kth[:, :, 128 * j:128 * (j + 1)],
                            kgx[g][H][BK * c:BK * (c + 1), coff:coff + 128]
                            .rearrange("(k p) q -> p k q", p=128),
                        )
                return ktA, ktB

            # blocks 0/1 kt tiles rotate into the wv buffers right here:
            # their loads run during the Q projection.
            kt0 = emit_kt(0)
            kt1 = emit_kt(1)

            # Wq^T halves rotate into Wk^T's buffers (dep: K matmuls done).
            wqA = wkq.tile([128, 8, D], BF16, tag="wh", name="wqA")
            wqB = wkq.tile([128, 8, D], BF16, tag="wh", name="wqB")
            nc.scalar.dma_start(
                wqA[:], wq_in[0:1024, :].rearrange("(k p) d -> p k d", p=128))
            nc.scalar.dma_start(
                wqB[:], wq_in[1024:2048, :].rearrange("(k p) d -> p k d",
                                                      p=128))

            # ------- phase Q: local Q^T projection ------
            for dp in range(8):
                psA = psum.tile([128, BK], F32, tag="pw", bufs=3,
                                name=f"psqA{dp}")
                psB = psum.tile([128, BK], F32, tag="pw", bufs=3,
                                name=f"psqB{dp}")
                for ki in range(16):
                    wqh = wqA if ki < 8 else wqB
                    nc.tensor.matmul(
                        psA[:, 0:SB], lhsT=wqh[:, ki % 8, 256 * dp:
                                               256 * dp + 128],
                        rhs=xq_sb[:, ki, :],
                        start=(ki == 0), stop=(ki == 15),
                    )
                    nc.tensor.matmul(
                        psB[:, 0:SB], lhsT=wqh[:, ki % 8, 256 * dp + 128:
                                               256 * dp + 256],
                        rhs=xq_sb[:, ki, :],
                        start=(ki == 0), stop=(ki == 15),
                    )
                nc.vector.tensor_copy(qt_sb[:, 2 * dp, :], psA[:, 0:SB])
                nc.vector.tensor_copy(qt_sb[:, 2 * dp + 1, :],
                                      psB[:, 0:SB])
                if dp == 1:
                    kq1_write(1)
                elif dp == 3:
                    vq_write(2, 0, vd2[0])
                    vq_write(3, 0, vd3[0])
                    nc.gpsimd.collective_compute(
                        "AllGather", ALU.bypass, replica_groups=RG,
                        ins=[vq_in[1][0].opt()], outs=[vgh[1][0].opt()],
                    )
                elif dp == 5:
                    vq_write(2, 1, vd2[1])
                    vq_write(3, 1, vd3[1])
                    nc.gpsimd.collective_compute(
                        "AllGather", ALU.bypass, replica_groups=RG,
                        ins=[vq_in[1][1].opt()], outs=[vgh[1][1].opt()],
                    )
            qes.close()

            # ---------------- attention (software-pipelined) ----------------
            accp = es.enter_context(tc.tile_pool(name="accp", bufs=1))
            vtl = es.enter_context(tc.tile_pool(name="vtl", bufs=4))
            mkl = es.enter_context(tc.tile_pool(name="mkl", bufs=3))
            pwork = es.enter_context(tc.tile_pool(name="pwork", bufs=2))

            acc = [accp.tile([128, D], F32, name=f"acc{t}") for t in range(4)]

            # Dropout masks load lazily: 3 upfront, then pair i+3's mask is
            # emitted right after pair i's pm frees its ring slot, so the
            # scalar-queue trigger NEVER waits (a waiting mask trigger
            # head-of-line-blocks the pex activations: measured 30us stall).
            PAIRLIST = [(B, s) for B in range(NBIG) for s in range(B, 4)]
            mk_all = {}

            def emit_mask(i):
                if i >= len(PAIRLIST):
                    return
                B, slot = PAIRLIST[i]
                mk = mkl.tile([128, BK], BF16, tag="mk",
                              name=f"mk{B}_{slot}")
                nc.scalar.dma_start(
                    mk[:],
                    mask_in[128 * slot:128 * (slot + 1),
                            BK * B:BK * (B + 1)],
                )
                mk_all[(B, slot)] = mk

            for i in range(3):
                emit_mask(i)

            def emit_vt_half(B, h, eng):
                vt = vtl.tile([128, 8, BK], BF16, tag="vt",
                              name=f"vt{'AB'[h]}{B}")
                for j in range(8):
                    r0 = VROW2[8 * B + j]
                    eng.dma_start(vt[:, j, :], vgh[B // 2][h][r0:r0 + 128, :])
                return vt

            def normalize_slot(slot):
                obf = pwork.tile([128, D], BF16, tag="obf", bufs=1,
                                 name=f"obf{slot}")
                nc.vector.tensor_reduce(
                    den[:, slot:slot + 1],
                    partials[:, PBASE[slot]:PBASE[slot] + KBMAX[slot]],
                    axis=mybir.AxisListType.X, op=ALU.add,
                )
                nc.vector.reciprocal(rec[:, slot:slot + 1],
                                     den[:, slot:slot + 1])
                nc.vector.tensor_scalar_mul(
                    obf[:], acc[slot][:], rec[:, slot:slot + 1])
                nc.scalar.dma_start(
                    out_ext[128 * slot:128 * (slot + 1), :], obf[:])

            def tp_stage(st):
                pm, vtA, vtB, B, slot = st
                pmt = pwork.tile([128, 8, 128], BF16, tag="pmt", bufs=3,
                                 name=f"pmt{B}_{slot}")
                for j in range(8):
                    tp = psum.tile([128, 128], BF16, tag="tp", bufs=2,
                                   name=f"tp{B}_{slot}_{j}")
                    nc.tensor.matmul(
                        tp[:], lhsT=pm[:, 128 * j:128 * (j + 1)],
                        rhs=ident_sb[:], is_transpose=True,
                        skip_group_check=True)
                    nc.scalar.copy(pmt[:, j, :], tp[:])
                return pmt

            def av_stage(st, pmt):
                pm, vtA, vtB, B, slot = st
                for h, vt in ((0, vtA), (1, vtB)):
                    av = psum.tile([128, BK], F32, tag="pw", bufs=3,
                                   name=f"av{B}_{slot}_{h}")
                    for j in range(8):
                        for n2 in range(2):
                            nc.tensor.matmul(
                                av[:, 512 * n2:512 * (n2 + 1)],
                                lhsT=pmt[:, j, :],
                                rhs=vt[:, j, 512 * n2:512 * (n2 + 1)],
                                start=(j == 0), stop=(j == 7),
                                skip_group_check=True,
                            )
                    if B == 0:
                        nc.vector.tensor_copy(
                            acc[slot][:, BK * h:BK * (h + 1)], av[:])
                    else:
                        nc.vector.scalar_tensor_tensor(
                            out=acc[slot][:, BK * h:BK * (h + 1)],
                            in0=av[:], scalar=1.0,
                            in1=acc[slot][:, BK * h:BK * (h + 1)],
                            op0=ALU.mult, op1=ALU.add,
                        )

            # software pipeline state: pair p's P-transposes run during
            # pair p+1's scores; its attn@V runs after pair p+2's scores
            # (the 2-pair lag lets the vt loads finish behind the Q-end
            # SBUF release without stalling the PE).
            state = {"prev": None, "prev_pmt": None, "old": None,
                     "old_pmt": None}

            def retire_old():
                if state["old"] is not None:
                    av_stage(state["old"], state["old_pmt"])
                    oB, oslot = state["old"][3], state["old"][4]
                    if oB == oslot:
                        # slot oslot's accumulation is complete (its
                        # diagonal block was its last): normalize and
                        # write it out now, hidden under later pairs.
                        normalize_slot(oslot)

            def emit_pairs(B, ktA, ktB, vtA, vtB):
                for slot in range(B, 4):
                    p = PBASE[slot] + B
                    mk = mk_all[(B, slot)]
                    sc = psum.tile([128, BK], F32, tag="pw", bufs=3,
                                   name=f"sc{B}_{slot}")
                    for ki in range(16):
                        if ki == 8 and state["prev"] is not None:
                            # interleave prev pair's P-transposes here so
                            # the pmt copies finish before its attn@V
                            state["prev_pmt"] = tp_stage(state["prev"])
                        kth = ktA if ki < 8 else ktB
                        for n2 in range(2):
                            nc.tensor.matmul(
                                sc[:, 512 * n2:512 * (n2 + 1)],
                                lhsT=qt_sb[:, ki, 128 * slot:128 * (slot + 1)],
                                rhs=kth[:, ki % 8, 512 * n2:512 * (n2 + 1)],
                                start=(ki == 0), stop=(ki == 15),
                                skip_group_check=True,
                            )
                    pex = pwork.tile([128, BK], BF16, tag="pex", bufs=1,
                                     name=f"pex{B}_{slot}")
                    nc.scalar.activation(pex[:], sc[:], AFT.Exp, scale=SCALE)
                    pcs = pwork.tile([128, BK], BF16, tag="pcs", bufs=1,
                                     name=f"pcs{B}_{slot}")
                    nc.vector.scalar_tensor_tensor(
                        out=pcs[:], in0=iota_sb[:],
                        scalar=sched_sb[:, p:p + 1], in1=pex[:],
                        op0=ALU.is_ge, op1=ALU.mult,
                        accum_out=partials[:, p:p + 1],
                    )
                    pm = pwork.tile([128, BK], BF16, tag="pm", bufs=3,
                                    name=f"pm{B}_{slot}")
                    nc.vector.tensor_mul(pm[:], pcs[:], mk[:])
                    emit_mask(PAIRLIST.index((B, slot)) + 3)
                    retire_old()
                    state["old"] = state["prev"]
                    state["old_pmt"] = state["prev_pmt"]
                    state["prev"] = (pm, vtA, vtB, B, slot)

            # group 1: blocks 0 and 1 (kt tiles already loading since the
            # V phase ended; vt tiles load from attention start).
            vtA0 = emit_vt_half(0, 0, nc.sync)
            vtB0 = emit_vt_half(0, 1, nc.gpsimd)
            vtA1 = emit_vt_half(1, 0, nc.sync)
            vtB1 = emit_vt_half(1, 1, nc.gpsimd)

            emit_pairs(0, kt0[0], kt0[1], vtA0, vtB0)
            emit_pairs(1, kt1[0], kt1[1], vtA1, vtB1)

            # group 2: blocks 2 and 3.  vt loads go on gpsimd — safe here
            # because no CC trigger is emitted after them.
            kt2 = emit_kt(2)
            kt3 = emit_kt(3)
            vtA2 = emit_vt_half(2, 0, nc.sync)
            vtA3 = emit_vt_half(3, 0, nc.sync)
            vtB2 = emit_vt_half(2, 1, nc.gpsimd)
            vtB3 = emit_vt_half(3, 1, nc.gpsimd)
            emit_pairs(2, kt2[0], kt2[1], vtA2, vtB2)
            emit_pairs(3, kt3[0], kt3[1], vtA3, vtB3)

            retire_old()
            state["old"] = state["prev"]
            state["old_pmt"] = tp_stage(state["prev"])
            retire_old()   # retires (3,3), which also normalizes slot 3

    nc.compile()
    return nc


_NC_CACHE = None


def _get_nc():
    global _NC_CACHE
    if _NC_CACHE is None:
        _NC_CACHE = build()
    return _NC_CACHE


def make_in_maps(x, Wq, Wk, Wv, drop_mask):
    bf = ml_dtypes.bfloat16
    x = np.asarray(x, dtype=np.float32)
    Wq = np.asarray(Wq, dtype=np.float32)
    Wk = np.asarray(Wk, dtype=np.float32)
    Wv = np.asarray(Wv, dtype=np.float32)
    drop_mask = np.asarray(drop_mask, dtype=np.float32)

    xT = np.ascontiguousarray(x.T).astype(bf)           # [D, S]
    wqT = np.ascontiguousarray(Wq.T.astype(bf))         # [D, D]
    wvT = np.ascontiguousarray(Wv.T.astype(bf))         # [D, D]
    wkT = np.ascontiguousarray(Wk.T.astype(bf))         # [D, D]
    mask_bf = drop_mask.astype(bf)

    in_maps = []
    for c in range(NC):
        tl = owned_tiles(c)
        thr = np.array(
            [1024.0 * B - 128.0 * tl[slot]
             for slot in range(4) for B in range(KBMAX[slot])],
            dtype=np.float32,
        )
        in_maps.append({
            "xq": np.ascontiguousarray(
                np.concatenate([xT[:, 128 * t:128 * (t + 1)] for t in tl],
                               axis=1)),
            "wqT": wqT,
            "wvT": wvT,
            "wkT": wkT,
            "drop_mask": np.ascontiguousarray(
                np.concatenate(
                    [mask_bf[128 * t:128 * (t + 1)] for t in tl], axis=0)),
            "sched": np.ascontiguousarray(np.tile(thr[None, :], (128, 1))),
        })
    return in_maps


def assemble(results):
    full = np.zeros((S, D), dtype=np.float32)
    for c in range(NC):
        o = np.asarray(results[c]["out"], dtype=np.float32)
        for slot, t in enumerate(owned_tiles(c)):
            full[128 * t:128 * (t + 1)] = o[128 * slot:128 * (slot + 1)]
    return full


def kernel(x, Wq, Wk, Wv, drop_mask):
    nc = _get_nc()
    in_maps = make_in_maps(x, Wq, Wk, Wv, drop_mask)
    res = bass_utils.run_bass_kernel_spmd(nc, in_maps, core_ids=list(range(NC)))
    return assemble(res.results)


def kernel_profiled(x, Wq, Wk, Wv, drop_mask):
    """Like kernel(), but captures an NTFF profile; returns (out, exec_time_ns,
    trace_path)."""
    nc = _get_nc()
    in_maps = make_in_maps(x, Wq, Wk, Wv, drop_mask)
    res = bass_utils.run_bass_kernel_spmd(
        nc, in_maps, core_ids=list(range(NC)), trace=True)
    trace_path = None
    if res.instructions_and_trace is not None:
        trace_path = res.instructions_and_trace[1]
    return assemble(res.results), res.exec_time_ns, trace_path


# revision 81
# speedup vs baseline: 1.0454x; 1.0009x over previous
"""Distributed causal-attention-with-dropout kernel for 8 TRN2 NeuronCores, v19.

Architecture ("all-local projections", fully static SPMD graph):

- Host pre-formats inputs (layout only, all model FLOPs stay on device):
  each core receives xq = x^T columns of its 4 OWNED q-tiles
  {c, 15-c, 16+c, 31-c} (bf16), the FULL Wq^T / Wk^T / Wv^T (bf16,
  replicated), dropout-mask rows of its owned tiles (bf16), and the causal
  threshold table.  There is NO x gather at all.
- A tiny dummy AllGather with NO input dependency fires at t~0 so the
  collectives-runtime init barrier (~20+40us) overlaps the K projection.
- Tensor phase order: K proj -> V slots {0,1} -> V slots {2,3} -> Q proj
  -> attention.  Startup loads are interleaved (wk ki-chunk, xq ki-chunk)
  across all 3 DMA queues and the K projection consumes ki in arrival
  order, so the PE starts at ~4us and stays dense (HAM un-throttled).
- K^T is AllGathered at FULL d_out depth in k-tile-group chunks, with
  group 0 further split into d_out halves so the first AG fires mid-K-proj:
  CC chain = KAG0a (4MB, after dp3), KAG0b (4MB), VAG0, KAG1 (8MB),
  VAG1 — exactly the attention consumption order, with no CC idle.
- Score big-block B needs only kg group B//2; block tiles share slot
  parity so each block reads one 128-column stripe of the AG output.
- kt tiles for blocks 0/1 load DURING the Q projection (the wv weight
  pool closes after V, freeing SBUF for ktl).  The KAG1/VAG1 triggers are
  EMITTED after the B=0/1 loads+pairs: DRAM DMAs conservatively wait on
  in-flight collectives, and CC-completion-dependent DMAs must NEVER sit
  on the gpsimd queue (they'd block the later CC triggers behind them).
- Attention: k-blocks are 1024 wide -> ZERO-padding static schedule
  (slot s needs exactly s+1 blocks; 10 pairs).  Causality enforced
  per-row by (iota(p-j) >= thr) * P on the vector engine; softmax without
  max-subtraction; denominators use pre-dropout sums.  Pair p's
  P-transposes and attn@V run after pair p+1's score matmuls.
"""

import math
import os
import sys
from contextlib import ExitStack

import numpy as np
import ml_dtypes

for _p in ("/opt/trn_rl_repo", "/root/.axon_site/_ro/trn_rl_repo"):
    if os.path.isdir(_p) and _p not in sys.path:
        sys.path.append(_p)

import concourse.bass as bass
import concourse.tile as tile
from concourse import bacc, mybir
from concourse import bass_utils
from concourse.masks import make_identity

S, D = 4096, 2048
NC = 8
SB = 512          # seq rows per core (4 owned 128-tiles)
BK = 1024         # big k-block width
NBIG = 4
KBMAX = (1, 2, 3, 4)
PBASE = (0, 1, 3, 6)
NPAIR = 10
SCALE = 1.0 / math.sqrt(float(D))
F32 = mybir.dt.float32
BF16 = mybir.dt.bfloat16
RG = [list(range(NC))]
ALU = mybir.AluOpType
AFT = mybir.ActivationFunctionType

# ki consumption order for the K projection: matches the 3-queue load
# arrival pattern (sync: ki 0..4, gpsimd: ki 5..9, scalar: ki 10..15) so
# the first dp group issues matmuls as chunks land.
KI_ORDER = (0, 5, 10, 1, 6, 11, 2, 7, 12, 3, 8, 13, 4, 9, 14, 15)


def owned_tiles(c):
    return (c, 15 - c, 16 + c, 31 - c)


def tile_owner_slot(t):
    if t <= 7:
        return t, 0
    if t <= 15:
        return 15 - t, 1
    if t <= 23:
        return t - 16, 2
    return 31 - t, 3


# row of tile t inside its V AllGather chunk (chunk = t//16; within a
# chunk, rank blocks of 256 rows hold slots {0,1} or {2,3})
VROW2 = [256 * tile_owner_slot(t)[0] + 128 * (tile_owner_slot(t)[1] % 2)
         for t in range(32)]


def build():
    nc = bacc.Bacc("TRN2", target_bir_lowering=False, debug=False,
                   num_devices=NC)

    xq_in = nc.dram_tensor("xq", [D, SB], BF16, kind="ExternalInput").ap()
    wq_in = nc.dram_tensor("wqT", [D, D], BF16, kind="ExternalInput").ap()
    wv_in = nc.dram_tensor("wvT", [D, D], BF16, kind="ExternalInput").ap()
    wk_in = nc.dram_tensor("wkT", [D, D], BF16, kind="ExternalInput").ap()
    mask_in = nc.dram_tensor("drop_mask", [4 * 128, S], BF16,
                             kind="ExternalInput").ap()
    sched_in = nc.dram_tensor("sched", [128, NPAIR], F32,
                              kind="ExternalInput").ap()
    out_ext = nc.dram_tensor("out", [4 * 128, D], BF16,
                             kind="ExternalOutput").ap()

    with tile.TileContext(nc) as tc:
        with ExitStack() as es:
            dram = es.enter_context(tc.tile_pool(name="dram", bufs=1,
                                                 space="DRAM"))
            const = es.enter_context(tc.tile_pool(name="const", bufs=1))
            psum = es.enter_context(tc.tile_pool(name="psum", bufs=1,
                                                 space="PSUM"))

            # ---------------- DRAM scratch ----------------
            dummy_in = dram.tile([1, NPAIR], F32, name="dummy_in")
            dummy_out = dram.tile([NC, NPAIR], F32, addr_space="Shared",
                                  name="dummy_out")
            # V contributions split by slot-pair (g=0: slots {0,1} = tiles
            # 0..15; g=1: slots {2,3}) AND by d_out half (h), giving four
            # 4MB AllGathers; vtA/vtB tiles map 1:1 onto the h-chunks.
            vq_in = [[dram.tile([256, BK], BF16, name=f"vq_in{g}_{h}")
                      for h in range(2)] for g in range(2)]
            vgh = [[dram.tile([NC * 256, BK], BF16, addr_space="Shared",
                              name=f"vg{g}_{h}") for h in range(2)]
                   for g in range(2)]
            # per-core K^T contributions at FULL d_out depth: group 0
            # (slots {0,1}) split into d_out halves (two 4MB AGs fired
            # during the K projection), group 1 as ONE 8MB AG whose input
            # is deliberately written only after V slot 0 — the CC core
            # runs whichever pending op triggered earliest, and ops whose
            # triggers tie race differently across cores (device crash),
            # so every AG gets a distinct compute-gated ready time.
            kq = [[dram.tile([BK, 256], BF16, name=f"kq{g}_{H}")
                   for H in range(2)] for g in range(2)]
            kgx = [[dram.tile([NC * BK, 256], BF16, addr_space="Shared",
                              name=f"kg{g}_{H}") for H in range(2)]
                   for g in range(2)]

            # dummy AllGather first, with NO input dependency (dummy_in is
            # never written): the CC trigger fires at t~0 so the
            # collectives-init barrier overlaps the K projection.
            nc.gpsimd.collective_compute(
                "AllGather", ALU.bypass, replica_groups=RG,
                ins=[dummy_in.opt()], outs=[dummy_out.opt()],
            )

            # ---------------- weight / activation loads ----------------
            sched_sb = const.tile([128, NPAIR], F32, name="sched_sb")
            nc.sync.dma_start(sched_sb[:], sched_in)

            att = es.enter_context(tc.tile_pool(name="att", bufs=1))
            qt_sb = att.tile([128, 16, SB], BF16, name="qt_sb")

            # Shared ring: the four Wv quarters ([128, 8, 1024] each, by
            # ki-half x d_out-half) and the attention kt tiles have the
            # SAME shape, so kt tiles rotate directly into the wv buffers
            # the moment the V projection stops reading them — kt loads
            # start ~70us before the Q projection finishes, costing zero
            # extra SBUF.  Lives until the end of the kernel.
            wvkt = es.enter_context(tc.tile_pool(name="wvkt", bufs=4))
            wvq = [[wvkt.tile([128, 8, BK], BF16, tag="wvkt",
                              name=f"wv{ih}_{h}") for h in range(2)]
                   for ih in range(2)]

            qes = ExitStack()
            xqp = qes.enter_context(tc.tile_pool(name="xqp", bufs=1))
            xq_sb = xqp.tile([128, 16, SB], BF16, name="xq_sb")
            stagep = qes.enter_context(tc.tile_pool(name="stagep", bufs=2))
            # wk pool: wqA/wqB rotate into wkA/wkB's buffers once the K
            # projection finishes.
            wkq = qes.enter_context(tc.tile_pool(name="wkq", bufs=2))

            wkA = wkq.tile([128, 8, D], BF16, tag="wh", name="wkA")
            wkB = wkq.tile([128, 8, D], BF16, tag="wh", name="wkB")

            # fine-grained interleaved startup loads across all 3 queues:
            # sync: ki 0..4, gpsimd: ki 5..9, scalar: ki 10..15, then
            # scalar continues with the wv quarters (wq + masks later).
            def wkx_chunk(eng, ki):
                wdst = wkA if ki < 8 else wkB
                eng.dma_start(wdst[:, ki % 8, :],
                              wk_in[128 * ki:128 * (ki + 1), :])
                eng.dma_start(xq_sb[:, ki, :],
                              xq_in[128 * ki:128 * (ki + 1), :])

            for ki in range(5):
                wkx_chunk(nc.sync, ki)
            for ki in range(5, 10):
                wkx_chunk(nc.gpsimd, ki)
            for ki in range(10, 16):
                wkx_chunk(nc.scalar, ki)
            for ih in range(2):
                for h in range(2):
                    nc.scalar.dma_start(
                        wvq[ih][h][:],
                        wv_in[1024 * ih:1024 * (ih + 1),
                              1024 * h:1024 * (h + 1)]
                        .rearrange("(k p) d -> p k d", p=128))

            # ---------------- constants (gpsimd engine, after triggers) ---
            ident_sb = const.tile([128, 128], BF16, name="ident_sb")
            make_identity(nc, ident_sb[:])
            iota_sb = const.tile([128, BK], F32, name="iota_sb")
            nc.gpsimd.iota(
                iota_sb[:], pattern=[[-1, BK]], base=0, channel_multiplier=1,
                allow_small_or_imprecise_dtypes=True,
            )

            partials = const.tile([128, NPAIR], F32, name="partials")
            den = const.tile([128, 4], F32, name="den")
            rec = const.tile([128, 4], F32, name="rec")

            # ------- phase K: local K^T (all d_out, own q) -------
            # kst cols 0:256 -> kq0 (written immediately, AGs fire at dp3
            # and dp7); cols 256:512 are parked in SBUF (k1all) and only
            # written to kq1 after V slot 0, spacing KAG1's trigger well
            # after KAG0b's and well before VAG0a's.
            k1all = stagep.tile([128, 16, 256], BF16, tag="k1all", bufs=1,
                                name="k1all")
            for dp in range(8):
                psA = psum.tile([128, BK], F32, tag="pw", bufs=3,
                                name=f"pskA{dp}")
                psB = psum.tile([128, BK], F32, tag="pw", bufs=3,
                                name=f"pskB{dp}")
                for i, ki in enumerate(KI_ORDER):
                    wkh = wkA if ki < 8 else wkB
                    nc.tensor.matmul(
                        psA[:, 0:SB], lhsT=wkh[:, ki % 8, 256 * dp:
                                               256 * dp + 128],
                        rhs=xq_sb[:, ki, :],
                        start=(i == 0), stop=(i == 15),
                    )
                    nc.tensor.matmul(
                        psB[:, 0:SB], lhsT=wkh[:, ki % 8, 256 * dp + 128:
                                               256 * dp + 256],
                        rhs=xq_sb[:, ki, :],
                        start=(i == 0), stop=(i == 15),
                    )
                for half, ps in ((0, psA), (1, psB)):
                    dt = 2 * dp + half
                    kst = stagep.tile([128, 256], BF16, tag="kst", bufs=6,
                                      name=f"kst{dt}")
                    nc.vector.tensor_copy(kst[:], ps[:, 0:256])
                    nc.vector.tensor_copy(k1all[:, dt, :], ps[:, 256:512])
                    nc.sync.dma_start(
                        kq[0][dt // 8][128 * (dt % 8):
                                       128 * (dt % 8) + 128, :],
                        kst[:])
                if dp in (3, 7):
                    H = dp // 4
                    nc.gpsimd.collective_compute(
                        "AllGather", ALU.bypass, replica_groups=RG,
                        ins=[kq[0][H].opt()], outs=[kgx[0][H].opt()],
                    )

            def kq1_write(H):
                for dt in range(8 * H, 8 * H + 8):
                    nc.sync.dma_start(
                        kq[1][H][128 * (dt % 8):128 * (dt % 8) + 128, :],
                        k1all[:, dt, :])
                nc.gpsimd.collective_compute(
                    "AllGather", ALU.bypass, replica_groups=RG,
                    ins=[kq[1][H].opt()], outs=[kgx[1][H].opt()],
                )

            # ------- phase V: natural layout, slots 0..3 ------
            # h=1 vq writes are DEFERRED two phases to ladder the V-gather
            # triggers ~17us apart (see kq/kg comment).
            def vq_write(st, h, vst):
                nc.sync.dma_start(
                    vq_in[st // 2][h]
                    .rearrange("(t p) d -> p t d", p=128)[:, st % 2, :],
                    vst[:])

            def v_slot(st, defer):
                deferred = []
                for h in range(2):
                    ps = psum.tile([128, BK], F32, tag="pw", bufs=3,
                                   name=f"psv{st}_{h}")
                    for ki in range(16):
                        for n2 in range(2):
                            nc.tensor.matmul(
                                ps[:, 512 * n2:512 * (n2 + 1)],
                                lhsT=xq_sb[:, ki, 128 * st:128 * (st + 1)],
                                rhs=wvq[ki // 8][h][
                                    :, ki % 8, 512 * n2:512 * (n2 + 1)],
                                start=(ki == 0), stop=(ki == 15),
                                skip_group_check=True,
                            )
                    vst = stagep.tile([128, BK], BF16, tag="vst", bufs=6,
                                      name=f"vst{st}_{h}")
                    nc.vector.tensor_copy(vst[:], ps[:])
                    if h in defer:
                        deferred.append(vst)
                    else:
                        vq_write(st, h, vst)
                return deferred

            # Chain order = consumption order; each op's input completes at
            # a distinct compute-gated point ~15us after the previous one:
            #   KAG0a (K dp3), KAG0b (K dp7), VAG0a (Vc1 st1-h0),
            #   VAG0b (after st2), KAG1a (after st3), KAG1b (Q dp1),
            #   VAG1a (Q dp3), VAG1b (Q dp5).
            vd0 = v_slot(0, defer={1})          # st0: h1 deferred
            vd1 = v_slot(1, defer={1})          # st1: h1 deferred
            nc.gpsimd.collective_compute(
                "AllGather", ALU.bypass, replica_groups=RG,
                ins=[vq_in[0][0].opt()], outs=[vgh[0][0].opt()],
            )
            vd2 = v_slot(2, defer={0, 1})       # st2: both deferred
            vq_write(0, 1, vd0[0])
            vq_write(1, 1, vd1[0])
            nc.gpsimd.collective_compute(
                "AllGather", ALU.bypass, replica_groups=RG,
                ins=[vq_in[0][1].opt()], outs=[vgh[0][1].opt()],
            )
            vd3 = v_slot(3, defer={0, 1})       # st3: both deferred
            kq1_write(0)

            # kt tiles: ktA = d_out rows 0..1023 (score ki 0..7), ktB =
            # rows 1024..2047, from kg0[H] (blocks 0/1) or kg1 (blocks 2/3).
            # All 8 tiles of a block share slot parity: one 128-col stripe.
            def emit_kt(B):
                ktA = wvkt.tile([128, 8, BK], BF16, tag="wvkt",
                                name=f"ktA{B}")
                ktB = wvkt.tile([128, 8, BK], BF16, tag="wvkt",
                                name=f"ktB{B}")
                g = B // 2
                coff = 128 * (B % 2)
                # Attention loads go on sync+gpsimd ONLY: scalar-engine DMA
                # triggers would head-of-line-block the pex activations
                # behind their AG waits (measured 57us pipeline collapse).
                for H, kth, eng in ((0, ktA, nc.sync), (1, ktB, nc.gpsimd)):
                    for j in range(8):
                        t = 8 * B + j
                        c, _s = tile_owner_slot(t)
                        eng.dma_start(
                            kth[:, :, 128 * j:128 * (j + 1)],
                            kgx[g][H][BK * c:BK * (c + 1), coff:coff + 128]
                            .rearrange("(k p) q -> p k q", p=128),
                        )
                return ktA, ktB

            # blocks 0/1 kt tiles rotate into the wv buffers right here:
            # their loads run during the Q projection.
            kt0 = emit_kt(0)
            kt1 = emit_kt(1)

            # Wq^T halves rotate into Wk^T's buffers (dep: K matmuls done).
            wqA = wkq.tile([128, 8, D], BF16, tag="wh", name="wqA")
            wqB = wkq.tile([128, 8, D], BF16, tag="wh", name="wqB")
            nc.scalar.dma_start(
                wqA[:], wq_in[0:1024, :].rearrange("(k p) d -> p k d", p=128))
            nc.scalar.dma_start(
                wqB[:], wq_in[1024:2048, :].rearrange("(k p) d -> p k d",
                                                      p=128))

            # ------- phase Q: local Q^T projection ------
            for dp in range(8):
                psA = psum.tile([128, BK], F32, tag="pw", bufs=3,
                                name=f"psqA{dp}")
                psB = psum.tile([128, BK], F32, tag="pw", bufs=3,
                                name=f"psqB{dp}")
                for ki in range(16):
                    wqh = wqA if ki < 8 else wqB
                    nc.tensor.matmul(
                        psA[:, 0:SB], lhsT=wqh[:, ki % 8, 256 * dp:
                                               256 * dp + 128],
                        rhs=xq_sb[:, ki, :],
                        start=(ki == 0), stop=(ki == 15),
                    )
                    nc.tensor.matmul(
                        psB[:, 0:SB], lhsT=wqh[:, ki % 8, 256 * dp + 128:
                                               256 * dp + 256],
                        rhs=xq_sb[:, ki, :],
                        start=(ki == 0), stop=(ki == 15),
                    )
                nc.vector.tensor_copy(qt_sb[:, 2 * dp, :], psA[:, 0:SB])
                nc.vector.tensor_copy(qt_sb[:, 2 * dp + 1, :],
                                      psB[:, 0:SB])
                if dp == 1:
                    kq1_write(1)
                elif dp == 3:
                    vq_write(2, 0, vd2[0])
                    vq_write(3, 0, vd3[0])
                    nc.gpsimd.collective_compute(
                        "AllGather", ALU.bypass, replica_groups=RG,
                        ins=[vq_in[1][0].opt()], outs=[vgh[1][0].opt()],
                    )
                elif dp == 5:
                    vq_write(2, 1, vd2[1])
                    vq_write(3, 1, vd3[1])
                    nc.gpsimd.collective_compute(
                        "AllGather", ALU.bypass, replica_groups=RG,
                        ins=[vq_in[1][1].opt()], outs=[vgh[1][1].opt()],
                    )
            qes.close()

            # ---------------- attention (software-pipelined) ----------------
            accp = es.enter_context(tc.tile_pool(name="accp", bufs=1))
            vtl = es.enter_context(tc.tile_pool(name="vtl", bufs=4))
            mkl = es.enter_context(tc.tile_pool(name="mkl", bufs=3))
            pwork = es.enter_context(tc.tile_pool(name="pwork", bufs=2))

            acc = [accp.tile([128, D], F32, name=f"acc{t}") for t in range(4)]

            # Dropout masks load lazily: 3 upfront, then pair i+3's mask is
            # emitted right after pair i's pm frees its ring slot, so the
            # scalar-queue trigger NEVER waits (a waiting mask trigger
            # head-of-line-blocks the pex activations: measured 30us stall).
            PAIRLIST = [(B, s) for B in range(NBIG) for s in range(B, 4)]
            mk_all = {}

            def emit_mask(i):
                if i >= len(PAIRLIST):
                    return
                B, slot = PAIRLIST[i]
                mk = mkl.tile([128, BK], BF16, tag="mk",
                              name=f"mk{B}_{slot}")
                nc.scalar.dma_start(
                    mk[:],
                    mask_in[128 * slot:128 * (slot + 1),
                            BK * B:BK * (B + 1)],
                )
                mk_all[(B, slot)] = mk

            for i in range(3):
                emit_mask(i)

            def emit_vt_half(B, h, eng):
                vt = vtl.tile([128, 8, BK], BF16, tag="vt",
                              name=f"vt{'AB'[h]}{B}")
                for j in range(8):
                    r0 = VROW2[8 * B + j]
                    eng.dma_start(vt[:, j, :], vgh[B // 2][h][r0:r0 + 128, :])
                return vt

            def normalize_slot(slot):
                obf = pwork.tile([128, D], BF16, tag="obf", bufs=1,
                                 name=f"obf{slot}")
                nc.vector.tensor_reduce(
                    den[:, slot:slot + 1],
                    partials[:, PBASE[slot]:PBASE[slot] + KBMAX[slot]],
                    axis=mybir.AxisListType.X, op=ALU.add,
                )
                nc.vector.reciprocal(rec[:, slot:slot + 1],
                                     den[:, slot:slot + 1])
                nc.vector.tensor_scalar_mul(
                    obf[:], acc[slot][:], rec[:, slot:slot + 1])
                nc.scalar.dma_start(
                    out_ext[128 * slot:128 * (slot + 1), :], obf[:])

            def tp_stage(st):
                pm, vtA, vtB, B, slot = st
                pmt = pwork.tile([128, 8, 128], BF16, tag="pmt", bufs=3,
                                 name=f"pmt{B}_{slot}")
                for j in range(8):
                    tp = psum.tile([128, 128], BF16, tag="tp", bufs=2,
                                   name=f"tp{B}_{slot}_{j}")
                    nc.tensor.matmul(
                        tp[:], lhsT=pm[:, 128 * j:128 * (j + 1)],
                        rhs=ident_sb[:], is_transpose=True,
                        skip_group_check=True)
                    nc.scalar.copy(pmt[:, j, :], tp[:])
                return pmt

            def av_stage(st, pmt):
                pm, vtA, vtB, B, slot = st
                for h, vt in ((0, vtA), (1, vtB)):
                    av = psum.tile([128, BK], F32, tag="pw", bufs=3,
                                   name=f"av{B}_{slot}_{h}")
                    for j in range(8):
                        for n2 in range(2):
                            nc.tensor.matmul(
                                av[:, 512 * n2:512 * (n2 + 1)],
                                lhsT=pmt[:, j, :],
                                rhs=vt[:, j, 512 * n2:512 * (n2 + 1)],
                                start=(j == 0), stop=(j == 7),
                                skip_group_check=True,
                            )
                    if B == 0:
                        nc.vector.tensor_copy(
                            acc[slot][:, BK * h:BK * (h + 1)], av[:])
                    else:
                        nc.vector.scalar_tensor_tensor(
                            out=acc[slot][:, BK * h:BK * (h + 1)],
                            in0=av[:], scalar=1.0,
                            in1=acc[slot][:, BK * h:BK * (h + 1)],
                            op0=ALU.mult, op1=ALU.add,
                        )

            # software pipeline state: pair p's P-transposes run during
            # pair p+1's scores; its attn@V runs after pair p+2's scores
            # (the 2-pair lag lets the vt loads finish behind the Q-end
            # SBUF release without stalling the PE).
            state = {"prev": None, "prev_pmt": None, "old": None,
                     "old_pmt": None}

            def retire_old():
                if state["old"] is not None:
                    av_stage(state["old"], state["old_pmt"])
                    oB, oslot = state["old"][3], state["old"][4]
                    if oB == oslot:
                        # slot oslot's accumulation is complete (its
                        # diagonal block was its last): normalize and
                        # write it out now, hidden under later pairs.
                        normalize_slot(oslot)

            def emit_pairs(B, ktA, ktB, vtA, vtB):
                for slot in range(B, 4):
                    p = PBASE[slot] + B
                    mk = mk_all[(B, slot)]
                    sc = psum.tile([128, BK], F32, tag="pw", bufs=3,
                                   name=f"sc{B}_{slot}")
                    for ki in range(16):
                        if ki == 8 and state["prev"] is not None:
                            # interleave prev pair's P-transposes here so
                            # the pmt copies finish before its attn@V
                            state["prev_pmt"] = tp_stage(state["prev"])
                        kth = ktA if ki < 8 else ktB
                        for n2 in range(2):
                            nc.tensor.matmul(
                                sc[:, 512 * n2:512 * (n2 + 1)],
                                lhsT=qt_sb[:, ki, 128 * slot:128 * (slot + 1)],
                                rhs=kth[:, ki % 8, 512 * n2:512 * (n2 + 1)],
                                start=(ki == 0), stop=(ki == 15),
                                skip_group_check=True,
                            )
                    pex = pwork.tile([128, BK], BF16, tag="pex", bufs=1,
                                     name=f"pex{B}_{slot}")
                    nc.scalar.activation(pex[:], sc[:], AFT.Exp, scale=SCALE)
                    pcs = pwork.tile([128, BK], BF16, tag="pcs", bufs=1,
                                     name=f"pcs{B}_{slot}")
                    nc.vector.scalar_tensor_tensor(
                        out=pcs[:], in0=iota_sb[:],
                        scalar=sched_sb[:, p:p + 1], in1=pex[:],
                        op0=ALU.is_ge, op1=ALU.mult,
                        accum_out=partials[:, p:p + 1],
                    )
                    pm = pwork.tile([128, BK], BF16, tag="pm", bufs=3,
                                    name=f"pm{B}_{slot}")
                    nc.vector.tensor_mul(pm[:], pcs[:], mk[:])
                    emit_mask(PAIRLIST.index((B, slot)) + 3)
                    retire_old()
                    state["old"] = state["prev"]
                    state["old_pmt"] = state["prev_pmt"]
                    state["prev"] = (pm, vtA, vtB, B, slot)

            # group 1: blocks 0 and 1 (kt tiles already loading since the
            # V phase ended; vt tiles load from attention start).
            vtA0 = emit_vt_half(0, 0, nc.sync)
            vtB0 = emit_vt_half(0, 1, nc.gpsimd)
            vtA1 = emit_vt_half(1, 0, nc.sync)
            vtB1 = emit_vt_half(1, 1, nc.gpsimd)

            # a short filler bridges the first pair's exp->mask->transpose
            # pipeline fill so the PE HAM throttle never re-engages.
            for w in range(12):
                tpw = psum.tile([128, 128], BF16, tag="tp", bufs=2,
                                name=f"warm{w}")
                nc.tensor.matmul(
                    tpw[:], lhsT=ident_sb[:], rhs=ident_sb[:],
                    is_transpose=True, skip_group_check=True)

            emit_pairs(0, kt0[0], kt0[1], vtA0, vtB0)
            emit_pairs(1, kt1[0], kt1[1], vtA1, vtB1)

            # group 2: blocks 2 and 3.  vt loads go on gpsimd — safe here
            # because no CC trigger is emitted after them.
            kt2 = emit_kt(2)
            kt3 = emit_kt(3)
            vtA2 = emit_vt_half(2, 0, nc.sync)
            vtA3 = emit_vt_half(3, 0, nc.sync)
            vtB2 = emit_vt_half(2, 1, nc.gpsimd)
            vtB3 = emit_vt_half(3, 1, nc.gpsimd)
            emit_pairs(2, kt2[0], kt2[1], vtA2, vtB2)
            emit_pairs(3, kt3[0], kt3[1], vtA3, vtB3)

            retire_old()
            state["old"] = state["prev"]
            state["old_pmt"] = tp_stage(state["prev"])
            retire_old()   # retires (3,3), which also normalizes slot 3

    nc.compile()
    return nc


_NC_CACHE = None


def _get_nc():
    global _NC_CACHE
    if _NC_CACHE is None:
        _NC_CACHE = build()
    return _NC_CACHE


def make_in_maps(x, Wq, Wk, Wv, drop_mask):
    bf = ml_dtypes.bfloat16
    x = np.asarray(x, dtype=np.float32)
    Wq = np.asarray(Wq, dtype=np.float32)
    Wk = np.asarray(Wk, dtype=np.float32)
    Wv = np.asarray(Wv, dtype=np.float32)
    drop_mask = np.asarray(drop_mask, dtype=np.float32)

    xT = np.ascontiguousarray(x.T).astype(bf)           # [D, S]
    wqT = np.ascontiguousarray(Wq.T.astype(bf))         # [D, D]
    wvT = np.ascontiguousarray(Wv.T.astype(bf))         # [D, D]
    wkT = np.ascontiguousarray(Wk.T.astype(bf))         # [D, D]
    mask_bf = drop_mask.astype(bf)

    in_maps = []
    for c in range(NC):
        tl = owned_tiles(c)
        thr = np.array(
            [1024.0 * B - 128.0 * tl[slot]
             for slot in range(4) for B in range(KBMAX[slot])],
            dtype=np.float32,
        )
        in_maps.append({
            "xq": np.ascontiguousarray(
                np.concatenate([xT[:, 128 * t:128 * (t + 1)] for t in tl],
                               axis=1)),
            "wqT": wqT,
            "wvT": wvT,
            "wkT": wkT,
            "drop_mask": np.ascontiguousarray(
                np.concatenate(
                    [mask_bf[128 * t:128 * (t + 1)] for t in tl], axis=0)),
            "sched": np.ascontiguousarray(np.tile(thr[None, :], (128, 1))),
        })
    return in_maps


def assemble(results):
    full = np.zeros((S, D), dtype=np.float32)
    for c in range(NC):
        o = np.asarray(results[c]["out"], dtype=np.float32)
        for slot, t in enumerate(owned_tiles(c)):
            full[128 * t:128 * (t + 1)] = o[128 * slot:128 * (slot + 1)]
    return full


def kernel(x, Wq, Wk, Wv, drop_mask):
    nc = _get_nc()
    in_maps = make_in_maps(x, Wq, Wk, Wv, drop_mask)
    res = bass_utils.run_bass_kernel_spmd(nc, in_maps, core_ids=list(range(NC)))
    return assemble(res.results)


def kernel_profiled(x, Wq, Wk, Wv, drop_mask):
    """Like kernel(), but captures an NTFF profile; returns (out, exec_time_ns,
    trace_path)."""
    nc = _get_nc()
    in_maps = make_in_maps(x, Wq, Wk, Wv, drop_mask)
    res = bass_utils.run_bass_kernel_spmd(
        nc, in_maps, core_ids=list(range(NC)), trace=True)
    trace_path = None
    if res.instructions_and_trace is not None:
        trace_path = res.instructions_and_trace[1]
    return assemble(res.results), res.exec_time_ns, trace_path


# revision 82
# speedup vs baseline: 1.0464x; 1.0009x over previous
"""Distributed causal-attention-with-dropout kernel for 8 TRN2 NeuronCores, v32.

Architecture ("all-local projections", fully static SPMD graph):

- Host pre-formats inputs (layout only, all model FLOPs stay on device):
  each core receives xq = x^T columns of its 4 OWNED q-tiles
  {c, 15-c, 16+c, 31-c} (bf16), the FULL Wq^T / Wk^T / Wv^T (bf16,
  replicated), dropout-mask rows of its owned tiles (bf16), and the causal
  threshold table.  There is NO x gather at all.
- A tiny dummy AllGather with NO input dependency fires at t~0 so the
  collectives-runtime init barrier (~20+20..100us, run-variable) overlaps
  the K projection.
- Tensor phase order: K proj -> V slots 0..3 -> Q proj -> attention.
  Startup loads are interleaved (wk ki-chunk, xq ki-chunk) across all 3
  DMA queues (HBM-read-bound, ~35us) and the K projection consumes ki in
  arrival order, so the PE starts at ~4us and stays dense (HAM throttle
  never re-engages; the PE sits at the 13/16 GPIO power cap, ~1.95GHz).
- Exchange = EIGHT 4MB AllGathers.  The CC core serially runs whichever
  pending op TRIGGERED first, and triggers fire when the op's input
  writes complete (DMA-queue descriptors, not engine-FIFO instructions),
  so chain order is enforced purely by a readiness LADDER: each op's
  input-completing write is deferred to a distinct compute-gated point
  ~15us after the previous one (ties race differently across cores and
  crash the device).  Chain = consumption order:
    KAG0a (K dp3), KAG0b (K dp7), VAG0a (V st1-h0), VAG0b (after st2),
    KAG1a (after st3), KAG1b (Q dp1), VAG1a (Q dp3), VAG1b (Q dp5),
  with kq1 columns parked in SBUF (k1all) and vq h1/st23 writes deferred
  via retained vst staging tiles.
- Score big-block B needs only kg group B//2 (full 2048-deep K^T per
  chunk); block tiles share slot parity so each block reads one 128-col
  stripe of the AG output.
- The four Wv quarters ([128, 8, 1024] by ki-half x d_out-half) and the
  attention kt tiles share one 4-buffer pool ring: kt tiles for blocks
  0/1 rotate into the wv buffers the moment V stops reading them, so
  their loads run DURING the Q projection at zero extra SBUF.
- Attention: k-blocks are 1024 wide -> ZERO-padding static schedule
  (slot s needs exactly s+1 blocks; 10 pairs).  Causality enforced
  per-row by (iota(p-j) >= thr) * P on the vector engine; softmax without
  max-subtraction; denominators use pre-dropout sums.  Software pipeline:
  pair p's P-transposes run during pair p+1's scores, its attn@V after
  pair p+2's scores (the 2-pair lag hides the vt load burst behind the
  Q-end SBUF release).  Each slot is normalized + written out (bf16) as
  soon as its diagonal block retires.
- Engine discipline: scalar runs pex/pmt/normalize compute, so NO DMA
  trigger that can wait (vt/kt loads) may sit on the scalar queue — a
  waiting trigger head-of-line-blocks pex and collapses the pipeline.
  Dropout masks load on scalar but lazily (3 ahead), emitted right after
  the pm that frees their ring slot, so their triggers never wait.
  Attention kt/vt loads go on sync+gpsimd; gpsimd is safe because every
  CC trigger is emitted before them.
"""

import math
import os
import sys
from contextlib import ExitStack

import numpy as np
import ml_dtypes

for _p in ("/opt/trn_rl_repo", "/root/.axon_site/_ro/trn_rl_repo"):
    if os.path.isdir(_p) and _p not in sys.path:
        sys.path.append(_p)

import concourse.bass as bass
import concourse.tile as tile
from concourse import bacc, mybir
from concourse import bass_utils
from concourse.masks import make_identity

S, D = 4096, 2048
NC = 8
SB = 512          # seq rows per core (4 owned 128-tiles)
BK = 1024         # big k-block width
NBIG = 4
KBMAX = (1, 2, 3, 4)
PBASE = (0, 1, 3, 6)
NPAIR = 10
SCALE = 1.0 / math.sqrt(float(D))
F32 = mybir.dt.float32
BF16 = mybir.dt.bfloat16
RG = [list(range(NC))]
ALU = mybir.AluOpType
AFT = mybir.ActivationFunctionType

# ki consumption order for the K projection: matches the 3-queue load
# arrival pattern (sync: ki 0..4, gpsimd: ki 5..9, scalar: ki 10..15) so
# the first dp group issues matmuls as chunks land.
KI_ORDER = (0, 5, 10, 1, 6, 11, 2, 7, 12, 3, 8, 13, 4, 9, 14, 15)


def owned_tiles(c):
    return (c, 15 - c, 16 + c, 31 - c)


def tile_owner_slot(t):
    if t <= 7:
        return t, 0
    if t <= 15:
        return 15 - t, 1
    if t <= 23:
        return t - 16, 2
    return 31 - t, 3


# row of tile t inside its V AllGather chunk (chunk = t//16; within a
# chunk, rank blocks of 256 rows hold slots {0,1} or {2,3})
VROW2 = [256 * tile_owner_slot(t)[0] + 128 * (tile_owner_slot(t)[1] % 2)
         for t in range(32)]


def build():
    nc = bacc.Bacc("TRN2", target_bir_lowering=False, debug=False,
                   num_devices=NC)

    xq_in = nc.dram_tensor("xq", [D, SB], BF16, kind="ExternalInput").ap()
    wq_in = nc.dram_tensor("wqT", [D, D], BF16, kind="ExternalInput").ap()
    wv_in = nc.dram_tensor("wvT", [D, D], BF16, kind="ExternalInput").ap()
    wk_in = nc.dram_tensor("wkT", [D, D], BF16, kind="ExternalInput").ap()
    mask_in = nc.dram_tensor("drop_mask", [4 * 128, S], BF16,
                             kind="ExternalInput").ap()
    sched_in = nc.dram_tensor("sched", [128, NPAIR], F32,
                              kind="ExternalInput").ap()
    out_ext = nc.dram_tensor("out", [4 * 128, D], BF16,
                             kind="ExternalOutput").ap()

    with tile.TileContext(nc) as tc:
        with ExitStack() as es:
            dram = es.enter_context(tc.tile_pool(name="dram", bufs=1,
                                                 space="DRAM"))
            const = es.enter_context(tc.tile_pool(name="const", bufs=1))
            psum = es.enter_context(tc.tile_pool(name="psum", bufs=1,
                                                 space="PSUM"))

            # ---------------- DRAM scratch ----------------
            dummy_in = dram.tile([1, NPAIR], F32, name="dummy_in")
            dummy_out = dram.tile([NC, NPAIR], F32, addr_space="Shared",
                                  name="dummy_out")
            # V contributions split by slot-pair (g=0: slots {0,1} = tiles
            # 0..15; g=1: slots {2,3}) AND by d_out half (h), giving four
            # 4MB AllGathers; vtA/vtB tiles map 1:1 onto the h-chunks.
            vq_in = [[dram.tile([256, BK], BF16, name=f"vq_in{g}_{h}")
                      for h in range(2)] for g in range(2)]
            vgh = [[dram.tile([NC * 256, BK], BF16, addr_space="Shared",
                              name=f"vg{g}_{h}") for h in range(2)]
                   for g in range(2)]
            # per-core K^T contributions at FULL d_out depth: group 0
            # (slots {0,1}) split into d_out halves (two 4MB AGs fired
            # during the K projection), group 1 as ONE 8MB AG whose input
            # is deliberately written only after V slot 0 — the CC core
            # runs whichever pending op triggered earliest, and ops whose
            # triggers tie race differently across cores (device crash),
            # so every AG gets a distinct compute-gated ready time.
            kq = [[dram.tile([BK, 256], BF16, name=f"kq{g}_{H}")
                   for H in range(2)] for g in range(2)]
            kgx = [[dram.tile([NC * BK, 256], BF16, addr_space="Shared",
                              name=f"kg{g}_{H}") for H in range(2)]
                   for g in range(2)]

            # dummy AllGather first, with NO input dependency (dummy_in is
            # never written): the CC trigger fires at t~0 so the
            # collectives-init barrier overlaps the K projection.
            nc.gpsimd.collective_compute(
                "AllGather", ALU.bypass, replica_groups=RG,
                ins=[dummy_in.opt()], outs=[dummy_out.opt()],
            )

            # ---------------- weight / activation loads ----------------
            sched_sb = const.tile([128, NPAIR], F32, name="sched_sb")
            nc.sync.dma_start(sched_sb[:], sched_in)

            att = es.enter_context(tc.tile_pool(name="att", bufs=1))
            qt_sb = att.tile([128, 16, SB], BF16, name="qt_sb")

            # Shared ring: the four Wv quarters ([128, 8, 1024] each, by
            # ki-half x d_out-half) and the attention kt tiles have the
            # SAME shape, so kt tiles rotate directly into the wv buffers
            # the moment the V projection stops reading them — kt loads
            # start ~70us before the Q projection finishes, costing zero
            # extra SBUF.  Lives until the end of the kernel.
            wvkt = es.enter_context(tc.tile_pool(name="wvkt", bufs=4))
            wvq = [[wvkt.tile([128, 8, BK], BF16, tag="wvkt",
                              name=f"wv{ih}_{h}") for h in range(2)]
                   for ih in range(2)]

            qes = ExitStack()
            xqp = qes.enter_context(tc.tile_pool(name="xqp", bufs=1))
            xq_sb = xqp.tile([128, 16, SB], BF16, name="xq_sb")
            stagep = qes.enter_context(tc.tile_pool(name="stagep", bufs=2))
            # wk pool: wqA/wqB rotate into wkA/wkB's buffers once the K
            # projection finishes.
            wkq = qes.enter_context(tc.tile_pool(name="wkq", bufs=2))

            wkA = wkq.tile([128, 8, D], BF16, tag="wh", name="wkA")
            wkB = wkq.tile([128, 8, D], BF16, tag="wh", name="wkB")

            # fine-grained interleaved startup loads across all 3 queues:
            # sync: ki 0..4, gpsimd: ki 5..9, scalar: ki 10..15, then
            # scalar continues with the wv quarters (wq + masks later).
            def wkx_chunk(eng, ki):
                wdst = wkA if ki < 8 else wkB
                eng.dma_start(wdst[:, ki % 8, :],
                              wk_in[128 * ki:128 * (ki + 1), :])
                eng.dma_start(xq_sb[:, ki, :],
                              xq_in[128 * ki:128 * (ki + 1), :])

            for ki in range(5):
                wkx_chunk(nc.sync, ki)
            for ki in range(5, 10):
                wkx_chunk(nc.gpsimd, ki)
            for ki in range(10, 16):
                wkx_chunk(nc.scalar, ki)
            for ih in range(2):
                for h in range(2):
                    nc.scalar.dma_start(
                        wvq[ih][h][:],
                        wv_in[1024 * ih:1024 * (ih + 1),
                              1024 * h:1024 * (h + 1)]
                        .rearrange("(k p) d -> p k d", p=128))

            # ---------------- constants (gpsimd engine, after triggers) ---
            ident_sb = const.tile([128, 128], BF16, name="ident_sb")
            make_identity(nc, ident_sb[:])
            iota_sb = const.tile([128, BK], F32, name="iota_sb")
            nc.gpsimd.iota(
                iota_sb[:], pattern=[[-1, BK]], base=0, channel_multiplier=1,
                allow_small_or_imprecise_dtypes=True,
            )

            partials = const.tile([128, NPAIR], F32, name="partials")
            den = const.tile([128, 4], F32, name="den")
            rec = const.tile([128, 4], F32, name="rec")

            # ------- phase K: local K^T (all d_out, own q) -------
            # kst cols 0:256 -> kq0 (written immediately, AGs fire at dp3
            # and dp7); cols 256:512 are parked in SBUF (k1all) and only
            # written to kq1 after V slot 0, spacing KAG1's trigger well
            # after KAG0b's and well before VAG0a's.
            k1all = stagep.tile([128, 16, 256], BF16, tag="k1all", bufs=1,
                                name="k1all")
            for dp in range(8):
                psA = psum.tile([128, BK], F32, tag="pw", bufs=3,
                                name=f"pskA{dp}")
                psB = psum.tile([128, BK], F32, tag="pw", bufs=3,
                                name=f"pskB{dp}")
                for i, ki in enumerate(KI_ORDER):
                    wkh = wkA if ki < 8 else wkB
                    nc.tensor.matmul(
                        psA[:, 0:SB], lhsT=wkh[:, ki % 8, 256 * dp:
                                               256 * dp + 128],
                        rhs=xq_sb[:, ki, :],
                        start=(i == 0), stop=(i == 15),
                    )
                    nc.tensor.matmul(
                        psB[:, 0:SB], lhsT=wkh[:, ki % 8, 256 * dp + 128:
                                               256 * dp + 256],
                        rhs=xq_sb[:, ki, :],
                        start=(i == 0), stop=(i == 15),
                    )
                for half, ps in ((0, psA), (1, psB)):
                    dt = 2 * dp + half
                    kst = stagep.tile([128, 256], BF16, tag="kst", bufs=6,
                                      name=f"kst{dt}")
                    nc.vector.tensor_copy(kst[:], ps[:, 0:256])
                    nc.vector.tensor_copy(k1all[:, dt, :], ps[:, 256:512])
                    nc.sync.dma_start(
                        kq[0][dt // 8][128 * (dt % 8):
                                       128 * (dt % 8) + 128, :],
                        kst[:])
                if dp in (3, 7):
                    H = dp // 4
                    nc.gpsimd.collective_compute(
                        "AllGather", ALU.bypass, replica_groups=RG,
                        ins=[kq[0][H].opt()], outs=[kgx[0][H].opt()],
                    )

            def kq1_write(H):
                for dt in range(8 * H, 8 * H + 8):
                    nc.sync.dma_start(
                        kq[1][H][128 * (dt % 8):128 * (dt % 8) + 128, :],
                        k1all[:, dt, :])
                nc.gpsimd.collective_compute(
                    "AllGather", ALU.bypass, replica_groups=RG,
                    ins=[kq[1][H].opt()], outs=[kgx[1][H].opt()],
                )

            # ------- phase V: natural layout, slots 0..3 ------
            # h=1 vq writes are DEFERRED two phases to ladder the V-gather
            # triggers ~17us apart (see kq/kg comment).
            def vq_write(st, h, vst):
                nc.sync.dma_start(
                    vq_in[st // 2][h]
                    .rearrange("(t p) d -> p t d", p=128)[:, st % 2, :],
                    vst[:])

            def v_slot(st, defer):
                deferred = []
                for h in range(2):
                    ps = psum.tile([128, BK], F32, tag="pw", bufs=3,
                                   name=f"psv{st}_{h}")
                    for ki in range(16):
                        for n2 in range(2):
                            nc.tensor.matmul(
                                ps[:, 512 * n2:512 * (n2 + 1)],
                                lhsT=xq_sb[:, ki, 128 * st:128 * (st + 1)],
                                rhs=wvq[ki // 8][h][
                                    :, ki % 8, 512 * n2:512 * (n2 + 1)],
                                start=(ki == 0), stop=(ki == 15),
                                skip_group_check=True,
                            )
                    vst = stagep.tile([128, BK], BF16, tag="vst", bufs=6,
                                      name=f"vst{st}_{h}")
                    nc.vector.tensor_copy(vst[:], ps[:])
                    if h in defer:
                        deferred.append(vst)
                    else:
                        vq_write(st, h, vst)
                return deferred

            # Chain order = consumption order; each op's input completes at
            # a distinct compute-gated point ~15us after the previous one:
            #   KAG0a (K dp3), KAG0b (K dp7), VAG0a (Vc1 st1-h0),
            #   VAG0b (after st2), KAG1a (after st3), KAG1b (Q dp1),
            #   VAG1a (Q dp3), VAG1b (Q dp5).
            vd0 = v_slot(0, defer={1})          # st0: h1 deferred
            vd1 = v_slot(1, defer={1})          # st1: h1 deferred
            nc.gpsimd.collective_compute(
                "AllGather", ALU.bypass, replica_groups=RG,
                ins=[vq_in[0][0].opt()], outs=[vgh[0][0].opt()],
            )
            vd2 = v_slot(2, defer={0, 1})       # st2: both deferred
            vq_write(0, 1, vd0[0])
            vq_write(1, 1, vd1[0])
            nc.gpsimd.collective_compute(
                "AllGather", ALU.bypass, replica_groups=RG,
                ins=[vq_in[0][1].opt()], outs=[vgh[0][1].opt()],
            )
            vd3 = v_slot(3, defer={0, 1})       # st3: both deferred
            kq1_write(0)

            # kt tiles: ktA = d_out rows 0..1023 (score ki 0..7), ktB =
            # rows 1024..2047, from kg0[H] (blocks 0/1) or kg1 (blocks 2/3).
            # All 8 tiles of a block share slot parity: one 128-col stripe.
            def emit_kt(B):
                ktA = wvkt.tile([128, 8, BK], BF16, tag="wvkt",
                                name=f"ktA{B}")
                ktB = wvkt.tile([128, 8, BK], BF16, tag="wvkt",
                                name=f"ktB{B}")
                g = B // 2
                coff = 128 * (B % 2)
                # Attention loads go on sync+gpsimd ONLY: scalar-engine DMA
                # triggers would head-of-line-block the pex activations
                # behind their AG waits (measured 57us pipeline collapse).
                for H, kth, eng in ((0, ktA, nc.sync), (1, ktB, nc.gpsimd)):
                    for j in range(8):
                        t = 8 * B + j
                        c, _s = tile_owner_slot(t)
                        eng.dma_start(
                            kth[:, :, 128 * j:128 * (j + 1)],
                            kgx[g][H][BK * c:BK * (c + 1), coff:coff + 128]
                            .rearrange("(k p) q -> p k q", p=128),
                        )
                return ktA, ktB

            # blocks 0/1 kt tiles rotate into the wv buffers right here:
            # their loads run during the Q projection.
            kt0 = emit_kt(0)
            kt1 = emit_kt(1)

            # Wq^T halves rotate into Wk^T's buffers (dep: K matmuls done).
            wqA = wkq.tile([128, 8, D], BF16, tag="wh", name="wqA")
            wqB = wkq.tile([128, 8, D], BF16, tag="wh", name="wqB")
            nc.scalar.dma_start(
                wqA[:], wq_in[0:1024, :].rearrange("(k p) d -> p k d", p=128))
            nc.scalar.dma_start(
                wqB[:], wq_in[1024:2048, :].rearrange("(k p) d -> p k d",
                                                      p=128))

            # ------- phase Q: local Q^T projection ------
            for dp in range(8):
                psA = psum.tile([128, BK], F32, tag="pw", bufs=3,
                                name=f"psqA{dp}")
                psB = psum.tile([128, BK], F32, tag="pw", bufs=3,
                                name=f"psqB{dp}")
                for ki in range(16):
                    wqh = wqA if ki < 8 else wqB
                    nc.tensor.matmul(
                        psA[:, 0:SB], lhsT=wqh[:, ki % 8, 256 * dp:
                                               256 * dp + 128],
                        rhs=xq_sb[:, ki, :],
                        start=(ki == 0), stop=(ki == 15),
                    )
                    nc.tensor.matmul(
                        psB[:, 0:SB], lhsT=wqh[:, ki % 8, 256 * dp + 128:
                                               256 * dp + 256],
                        rhs=xq_sb[:, ki, :],
                        start=(ki == 0), stop=(ki == 15),
                    )
                nc.vector.tensor_copy(qt_sb[:, 2 * dp, :], psA[:, 0:SB])
                nc.vector.tensor_copy(qt_sb[:, 2 * dp + 1, :],
                                      psB[:, 0:SB])
                if dp == 1:
                    kq1_write(1)
                elif dp == 3:
                    vq_write(2, 0, vd2[0])
                    vq_write(3, 0, vd3[0])
                    nc.gpsimd.collective_compute(
                        "AllGather", ALU.bypass, replica_groups=RG,
                        ins=[vq_in[1][0].opt()], outs=[vgh[1][0].opt()],
                    )
                elif dp == 5:
                    vq_write(2, 1, vd2[1])
                    vq_write(3, 1, vd3[1])
                    nc.gpsimd.collective_compute(
                        "AllGather", ALU.bypass, replica_groups=RG,
                        ins=[vq_in[1][1].opt()], outs=[vgh[1][1].opt()],
                    )
            qes.close()

            # ---------------- attention (software-pipelined) ----------------
            accp = es.enter_context(tc.tile_pool(name="accp", bufs=1))
            vtl = es.enter_context(tc.tile_pool(name="vtl", bufs=4))
            mkl = es.enter_context(tc.tile_pool(name="mkl", bufs=3))
            pwork = es.enter_context(tc.tile_pool(name="pwork", bufs=2))

            acc = [accp.tile([128, D], F32, name=f"acc{t}") for t in range(4)]

            # Dropout masks load lazily: 3 upfront, then pair i+3's mask is
            # emitted right after pair i's pm frees its ring slot, so the
            # scalar-queue trigger NEVER waits (a waiting mask trigger
            # head-of-line-blocks the pex activations: measured 30us stall).
            PAIRLIST = [(B, s) for B in range(NBIG) for s in range(B, 4)]
            mk_all = {}

            def emit_mask(i):
                if i >= len(PAIRLIST):
                    return
                B, slot = PAIRLIST[i]
                mk = mkl.tile([128, BK], BF16, tag="mk",
                              name=f"mk{B}_{slot}")
                nc.scalar.dma_start(
                    mk[:],
                    mask_in[128 * slot:128 * (slot + 1),
                            BK * B:BK * (B + 1)],
                )
                mk_all[(B, slot)] = mk

            for i in range(3):
                emit_mask(i)

            def emit_vt_half(B, h, eng):
                vt = vtl.tile([128, 8, BK], BF16, tag="vt",
                              name=f"vt{'AB'[h]}{B}")
                for j in range(8):
                    r0 = VROW2[8 * B + j]
                    eng.dma_start(vt[:, j, :], vgh[B // 2][h][r0:r0 + 128, :])
                return vt

            def normalize_slot(slot):
                obf = pwork.tile([128, D], BF16, tag="obf", bufs=1,
                                 name=f"obf{slot}")
                nc.vector.tensor_reduce(
                    den[:, slot:slot + 1],
                    partials[:, PBASE[slot]:PBASE[slot] + KBMAX[slot]],
                    axis=mybir.AxisListType.X, op=ALU.add,
                )
                nc.vector.reciprocal(rec[:, slot:slot + 1],
                                     den[:, slot:slot + 1])
                nc.vector.tensor_scalar_mul(
                    obf[:], acc[slot][:], rec[:, slot:slot + 1])
                nc.scalar.dma_start(
                    out_ext[128 * slot:128 * (slot + 1), :], obf[:])

            def tp_stage(st):
                pm, vtA, vtB, B, slot = st
                pmt = pwork.tile([128, 8, 128], BF16, tag="pmt", bufs=3,
                                 name=f"pmt{B}_{slot}")
                for j in range(8):
                    tp = psum.tile([128, 128], BF16, tag="tp", bufs=2,
                                   name=f"tp{B}_{slot}_{j}")
                    nc.tensor.matmul(
                        tp[:], lhsT=pm[:, 128 * j:128 * (j + 1)],
                        rhs=ident_sb[:], is_transpose=True,
                        skip_group_check=True)
                    nc.scalar.copy(pmt[:, j, :], tp[:])
                return pmt

            def av_stage(st, pmt):
                pm, vtA, vtB, B, slot = st
                for h, vt in ((0, vtA), (1, vtB)):
                    av = psum.tile([128, BK], F32, tag="pw", bufs=3,
                                   name=f"av{B}_{slot}_{h}")
                    for j in range(8):
                        for n2 in range(2):
                            nc.tensor.matmul(
                                av[:, 512 * n2:512 * (n2 + 1)],
                                lhsT=pmt[:, j, :],
                                rhs=vt[:, j, 512 * n2:512 * (n2 + 1)],
                                start=(j == 0), stop=(j == 7),
                                skip_group_check=True,
                            )
                    if B == 0:
                        nc.vector.tensor_copy(
                            acc[slot][:, BK * h:BK * (h + 1)], av[:])
                    else:
                        nc.vector.scalar_tensor_tensor(
                            out=acc[slot][:, BK * h:BK * (h + 1)],
                            in0=av[:], scalar=1.0,
                            in1=acc[slot][:, BK * h:BK * (h + 1)],
                            op0=ALU.mult, op1=ALU.add,
                        )

            # software pipeline state: pair p's P-transposes run during
            # pair p+1's scores; its attn@V runs after pair p+2's scores
            # (the 2-pair lag lets the vt loads finish behind the Q-end
            # SBUF release without stalling the PE).
            state = {"prev": None, "prev_pmt": None, "old": None,
                     "old_pmt": None}

            def retire_old():
                if state["old"] is not None:
                    av_stage(state["old"], state["old_pmt"])
                    oB, oslot = state["old"][3], state["old"][4]
                    if oB == oslot:
                        # slot oslot's accumulation is complete (its
                        # diagonal block was its last): normalize and
                        # write it out now, hidden under later pairs.
                        normalize_slot(oslot)

            def emit_pairs(B, ktA, ktB, vtA, vtB):
                for slot in range(B, 4):
                    p = PBASE[slot] + B
                    mk = mk_all[(B, slot)]
                    sc = psum.tile([128, BK], F32, tag="pw", bufs=3,
                                   name=f"sc{B}_{slot}")
                    for ki in range(16):
                        if ki == 8 and state["prev"] is not None:
                            # interleave prev pair's P-transposes here so
                            # the pmt copies finish before its attn@V
                            state["prev_pmt"] = tp_stage(state["prev"])
                        kth = ktA if ki < 8 else ktB
                        for n2 in range(2):
                            nc.tensor.matmul(
                                sc[:, 512 * n2:512 * (n2 + 1)],
                                lhsT=qt_sb[:, ki, 128 * slot:128 * (slot + 1)],
                                rhs=kth[:, ki % 8, 512 * n2:512 * (n2 + 1)],
                                start=(ki == 0), stop=(ki == 15),
                                skip_group_check=True,
                            )
                    pex = pwork.tile([128, BK], BF16, tag="pex", bufs=1,
                                     name=f"pex{B}_{slot}")
                    nc.scalar.activation(pex[:], sc[:], AFT.Exp, scale=SCALE)
                    pcs = pwork.tile([128, BK], BF16, tag="pcs", bufs=1,
                                     name=f"pcs{B}_{slot}")
                    nc.vector.scalar_tensor_tensor(
                        out=pcs[:], in0=iota_sb[:],
                        scalar=sched_sb[:, p:p + 1], in1=pex[:],
                        op0=ALU.is_ge, op1=ALU.mult,
                        accum_out=partials[:, p:p + 1],
                    )
                    pm = pwork.tile([128, BK], BF16, tag="pm", bufs=3,
                                    name=f"pm{B}_{slot}")
                    nc.vector.tensor_mul(pm[:], pcs[:], mk[:])
                    emit_mask(PAIRLIST.index((B, slot)) + 3)
                    retire_old()
                    state["old"] = state["prev"]
                    state["old_pmt"] = state["prev_pmt"]
                    state["prev"] = (pm, vtA, vtB, B, slot)

            # group 1: blocks 0 and 1 (kt tiles already loading since the
            # V phase ended; vt tiles load from attention start).
            vtA0 = emit_vt_half(0, 0, nc.sync)
            vtB0 = emit_vt_half(0, 1, nc.gpsimd)
            vtA1 = emit_vt_half(1, 0, nc.sync)
            vtB1 = emit_vt_half(1, 1, nc.gpsimd)

            # a short filler bridges the first pair's exp->mask->transpose
            # pipeline fill so the PE HAM throttle never re-engages.
            for w in range(12):
                tpw = psum.tile([128, 128], BF16, tag="tp", bufs=2,
                                name=f"warm{w}")
                nc.tensor.matmul(
                    tpw[:], lhsT=ident_sb[:], rhs=ident_sb[:],
                    is_transpose=True, skip_group_check=True)

            emit_pairs(0, kt0[0], kt0[1], vtA0, vtB0)
            emit_pairs(1, kt1[0], kt1[1], vtA1, vtB1)

            # group 2: blocks 2 and 3.  vt loads go on gpsimd — safe here
            # because no CC trigger is emitted after them.
            kt2 = emit_kt(2)
            kt3 = emit_kt(3)
            vtA2 = emit_vt_half(2, 0, nc.sync)
            vtA3 = emit_vt_half(3, 0, nc.sync)
            vtB2 = emit_vt_half(2, 1, nc.gpsimd)
            vtB3 = emit_vt_half(3, 1, nc.gpsimd)
            emit_pairs(2, kt2[0], kt2[1], vtA2, vtB2)
            emit_pairs(3, kt3[0], kt3[1], vtA3, vtB3)

            retire_old()
            state["old"] = state["prev"]
            state["old_pmt"] = tp_stage(state["prev"])
            retire_old()   # retires (3,3), which also normalizes slot 3

    nc.compile()
    return nc


_NC_CACHE = None


def _get_nc():
    global _NC_CACHE
    if _NC_CACHE is None:
        _NC_CACHE = build()
    return _NC_CACHE


def make_in_maps(x, Wq, Wk, Wv, drop_mask):
    bf = ml_dtypes.bfloat16
    x = np.asarray(x, dtype=np.float32)
    Wq = np.asarray(Wq, dtype=np.float32)
    Wk = np.asarray(Wk, dtype=np.float32)
    Wv = np.asarray(Wv, dtype=np.float32)
    drop_mask = np.asarray(drop_mask, dtype=np.float32)

    xT = np.ascontiguousarray(x.T).astype(bf)           # [D, S]
    wqT = np.ascontiguousarray(Wq.T.astype(bf))         # [D, D]
    wvT = np.ascontiguousarray(Wv.T.astype(bf))         # [D, D]
    wkT = np.ascontiguousarray(Wk.T.astype(bf))         # [D, D]
    mask_bf = drop_mask.astype(bf)

    in_maps = []
    for c in range(NC):
        tl = owned_tiles(c)
        thr = np.array(
            [1024.0 * B - 128.0 * tl[slot]
             for slot in range(4) for B in range(KBMAX[slot])],
            dtype=np.float32,
        )
        in_maps.append({
            "xq": np.ascontiguousarray(
                np.concatenate([xT[:, 128 * t:128 * (t + 1)] for t in tl],
                               axis=1)),
            "wqT": wqT,
            "wvT": wvT,
            "wkT": wkT,
            "drop_mask": np.ascontiguousarray(
                np.concatenate(
                    [mask_bf[128 * t:128 * (t + 1)] for t in tl], axis=0)),
            "sched": np.ascontiguousarray(np.tile(thr[None, :], (128, 1))),
        })
    return in_maps


def assemble(results):
    full = np.zeros((S, D), dtype=np.float32)
    for c in range(NC):
        o = np.asarray(results[c]["out"], dtype=np.float32)
        for slot, t in enumerate(owned_tiles(c)):
            full[128 * t:128 * (t + 1)] = o[128 * slot:128 * (slot + 1)]
    return full


def kernel(x, Wq, Wk, Wv, drop_mask):
    nc = _get_nc()
    in_maps = make_in_maps(x, Wq, Wk, Wv, drop_mask)
    res = bass_utils.run_bass_kernel_spmd(nc, in_maps, core_ids=list(range(NC)))
    return assemble(res.results)


def kernel_profiled(x, Wq, Wk, Wv, drop_mask):
    """Like kernel(), but captures an NTFF profile; returns (out, exec_time_ns,
    trace_path)."""
    nc = _get_nc()
    in_maps = make_in_maps(x, Wq, Wk, Wv, drop_mask)
    res = bass_utils.run_bass_kernel_spmd(
        nc, in_maps, core_ids=list(range(NC)), trace=True)
    trace_path = None
    if res.instructions_and_trace is not None:
        trace_path = res.instructions_and_trace[1]
    return assemble(res.results), res.exec_time_ns, trace_path


# revision 85
# speedup vs baseline: 1.0589x; 1.0120x over previous
"""Distributed causal-attention-with-dropout kernel for 8 TRN2 NeuronCores, v32.

Architecture ("all-local projections", fully static SPMD graph):

- Host pre-formats inputs (layout only, all model FLOPs stay on device):
  each core receives xq = x^T columns of its 4 OWNED q-tiles
  {c, 15-c, 16+c, 31-c} (bf16), the FULL Wq^T / Wk^T / Wv^T (bf16,
  replicated), dropout-mask rows of its owned tiles (bf16), and the causal
  threshold table.  There is NO x gather at all.
- A tiny dummy AllGather with NO input dependency fires at t~0 so the
  collectives-runtime init barrier (~20+20..100us, run-variable) overlaps
  the K projection.
- Tensor phase order: K proj -> V slots 0..3 -> Q proj -> attention.
  Startup loads are interleaved (wk ki-chunk, xq ki-chunk) across all 3
  DMA queues (HBM-read-bound, ~35us) and the K projection consumes ki in
  arrival order, so the PE starts at ~4us and stays dense (HAM throttle
  never re-engages; the PE sits at the 13/16 GPIO power cap, ~1.95GHz).
- Exchange = EIGHT 4MB AllGathers.  The CC core serially runs whichever
  pending op TRIGGERED first, and triggers fire when the op's input
  writes complete (DMA-queue descriptors, not engine-FIFO instructions),
  so chain order is enforced purely by a readiness LADDER: each op's
  input-completing write is deferred to a distinct compute-gated point
  ~15us after the previous one (ties race differently across cores and
  crash the device).  Chain = consumption order:
    KAG0a (K dp3), KAG0b (K dp7), VAG0a (V st1-h0), VAG0b (after st2),
    KAG1a (after st3), KAG1b (Q dp1), VAG1a (Q dp3), VAG1b (Q dp5),
  with kq1 columns parked in SBUF (k1all) and vq h1/st23 writes deferred
  via retained vst staging tiles.
- Score big-block B needs only kg group B//2 (full 2048-deep K^T per
  chunk); block tiles share slot parity so each block reads one 128-col
  stripe of the AG output.
- The four Wv quarters ([128, 8, 1024] by ki-half x d_out-half) and the
  attention kt tiles share one 4-buffer pool ring: kt tiles for blocks
  0/1 rotate into the wv buffers the moment V stops reading them, so
  their loads run DURING the Q projection at zero extra SBUF.
- Attention: k-blocks are 1024 wide -> ZERO-padding static schedule
  (slot s needs exactly s+1 blocks; 10 pairs).  Causality enforced
  per-row by (iota(p-j) >= thr) * P on the vector engine; softmax without
  max-subtraction; denominators use pre-dropout sums.  Software pipeline:
  pair p's P-transposes run during pair p+1's scores, its attn@V after
  pair p+2's scores (the 2-pair lag hides the vt load burst behind the
  Q-end SBUF release).  Each slot is normalized + written out (bf16) as
  soon as its diagonal block retires.
- Engine discipline: scalar runs pex/pmt/normalize compute, so NO DMA
  trigger that can wait (vt/kt loads) may sit on the scalar queue — a
  waiting trigger head-of-line-blocks pex and collapses the pipeline.
  Dropout masks load on scalar but lazily (3 ahead), emitted right after
  the pm that frees their ring slot, so their triggers never wait.
  Attention kt/vt loads go on sync+gpsimd; gpsimd is safe because every
  CC trigger is emitted before them.
"""

import math
import os
import sys
from contextlib import ExitStack

import numpy as np
import ml_dtypes

for _p in ("/opt/trn_rl_repo", "/root/.axon_site/_ro/trn_rl_repo"):
    if os.path.isdir(_p) and _p not in sys.path:
        sys.path.append(_p)

import concourse.bass as bass
import concourse.tile as tile
from concourse import bacc, mybir
from concourse import bass_utils
from concourse.masks import make_identity

S, D = 4096, 2048
NC = 8
SB = 512          # seq rows per core (4 owned 128-tiles)
BK = 1024         # big k-block width
NBIG = 4
KBMAX = (1, 2, 3, 4)
PBASE = (0, 1, 3, 6)
NPAIR = 10
SCALE = 1.0 / math.sqrt(float(D))
F32 = mybir.dt.float32
BF16 = mybir.dt.bfloat16
RG = [list(range(NC))]
ALU = mybir.AluOpType
AFT = mybir.ActivationFunctionType

# ki consumption order for the K projection: matches the 3-queue load
# arrival pattern (sync: ki 0..5, gpsimd: ki 6..11, scalar: ki 12..15 —
# scalar carries fewer because the 8MB of wv quarters follow it) so the
# first dp group issues matmuls as chunks land.
KI_ORDER = (0, 6, 12, 1, 7, 13, 2, 8, 14, 3, 9, 15, 4, 10, 5, 11)


def owned_tiles(c):
    return (c, 15 - c, 16 + c, 31 - c)


def tile_owner_slot(t):
    if t <= 7:
        return t, 0
    if t <= 15:
        return 15 - t, 1
    if t <= 23:
        return t - 16, 2
    return 31 - t, 3


# row of tile t inside its V AllGather chunk (chunk = t//16; within a
# chunk, rank blocks of 256 rows hold slots {0,1} or {2,3})
VROW2 = [256 * tile_owner_slot(t)[0] + 128 * (tile_owner_slot(t)[1] % 2)
         for t in range(32)]


def build():
    nc = bacc.Bacc("TRN2", target_bir_lowering=False, debug=False,
                   num_devices=NC)

    xq_in = nc.dram_tensor("xq", [D, SB], BF16, kind="ExternalInput").ap()
    wq_in = nc.dram_tensor("wqT", [D, D], BF16, kind="ExternalInput").ap()
    wv_in = nc.dram_tensor("wvT", [D, D], BF16, kind="ExternalInput").ap()
    wk_in = nc.dram_tensor("wkT", [D, D], BF16, kind="ExternalInput").ap()
    mask_in = nc.dram_tensor("drop_mask", [4 * 128, S], BF16,
                             kind="ExternalInput").ap()
    sched_in = nc.dram_tensor("sched", [128, NPAIR], F32,
                              kind="ExternalInput").ap()
    out_ext = nc.dram_tensor("out", [4 * 128, D], BF16,
                             kind="ExternalOutput").ap()

    with tile.TileContext(nc) as tc:
        with ExitStack() as es:
            dram = es.enter_context(tc.tile_pool(name="dram", bufs=1,
                                                 space="DRAM"))
            const = es.enter_context(tc.tile_pool(name="const", bufs=1))
            psum = es.enter_context(tc.tile_pool(name="psum", bufs=1,
                                                 space="PSUM"))

            # ---------------- DRAM scratch ----------------
            dummy_in = dram.tile([1, NPAIR], F32, name="dummy_in")
            dummy_out = dram.tile([NC, NPAIR], F32, addr_space="Shared",
                                  name="dummy_out")
            # V contributions split by slot-pair (g=0: slots {0,1} = tiles
            # 0..15; g=1: slots {2,3}) AND by d_out half (h), giving four
            # 4MB AllGathers; vtA/vtB tiles map 1:1 onto the h-chunks.
            vq_in = [[dram.tile([256, BK], BF16, name=f"vq_in{g}_{h}")
                      for h in range(2)] for g in range(2)]
            vgh = [[dram.tile([NC * 256, BK], BF16, addr_space="Shared",
                              name=f"vg{g}_{h}") for h in range(2)]
                   for g in range(2)]
            # per-core K^T contributions at FULL d_out depth: group 0
            # (slots {0,1}) split into d_out halves (two 4MB AGs fired
            # during the K projection), group 1 as ONE 8MB AG whose input
            # is deliberately written only after V slot 0 — the CC core
            # runs whichever pending op triggered earliest, and ops whose
            # triggers tie race differently across cores (device crash),
            # so every AG gets a distinct compute-gated ready time.
            kq = [[dram.tile([BK, 256], BF16, name=f"kq{g}_{H}")
                   for H in range(2)] for g in range(2)]
            kgx = [[dram.tile([NC * BK, 256], BF16, addr_space="Shared",
                              name=f"kg{g}_{H}") for H in range(2)]
                   for g in range(2)]

            # dummy AllGather first, with NO input dependency (dummy_in is
            # never written): the CC trigger fires at t~0 so the
            # collectives-init barrier overlaps the K projection.
            nc.gpsimd.collective_compute(
                "AllGather", ALU.bypass, replica_groups=RG,
                ins=[dummy_in.opt()], outs=[dummy_out.opt()],
            )

            # ---------------- weight / activation loads ----------------
            sched_sb = const.tile([128, NPAIR], F32, name="sched_sb")
            nc.sync.dma_start(sched_sb[:], sched_in)

            att = es.enter_context(tc.tile_pool(name="att", bufs=1))
            qt_sb = att.tile([128, 16, SB], BF16, name="qt_sb")

            # Shared ring: the four Wv quarters ([128, 8, 1024] each, by
            # ki-half x d_out-half) and the attention kt tiles have the
            # SAME shape, so kt tiles rotate directly into the wv buffers
            # the moment the V projection stops reading them — kt loads
            # start ~70us before the Q projection finishes, costing zero
            # extra SBUF.  Lives until the end of the kernel.
            wvkt = es.enter_context(tc.tile_pool(name="wvkt", bufs=4))
            wvq = [[wvkt.tile([128, 8, BK], BF16, tag="wvkt",
                              name=f"wv{ih}_{h}") for h in range(2)]
                   for ih in range(2)]

            qes = ExitStack()
            xqp = qes.enter_context(tc.tile_pool(name="xqp", bufs=1))
            xq_sb = xqp.tile([128, 16, SB], BF16, name="xq_sb")
            stagep = qes.enter_context(tc.tile_pool(name="stagep", bufs=2))
            # wk pool: wqA/wqB rotate into wkA/wkB's buffers once the K
            # projection finishes.
            wkq = qes.enter_context(tc.tile_pool(name="wkq", bufs=2))

            wkA = wkq.tile([128, 8, D], BF16, tag="wh", name="wkA")
            wkB = wkq.tile([128, 8, D], BF16, tag="wh", name="wkB")

            # fine-grained interleaved startup loads across all 3 queues:
            # sync: ki 0..4, gpsimd: ki 5..9, scalar: ki 10..15, then
            # scalar continues with the wv quarters (wq + masks later).
            def wkx_chunk(eng, ki):
                wdst = wkA if ki < 8 else wkB
                eng.dma_start(wdst[:, ki % 8, :],
                              wk_in[128 * ki:128 * (ki + 1), :])
                eng.dma_start(xq_sb[:, ki, :],
                              xq_in[128 * ki:128 * (ki + 1), :])

            for ki in range(6):
                wkx_chunk(nc.sync, ki)
            for ki in range(6, 12):
                wkx_chunk(nc.gpsimd, ki)
            for ki in range(12, 16):
                wkx_chunk(nc.scalar, ki)
            for ih in range(2):
                for h in range(2):
                    nc.scalar.dma_start(
                        wvq[ih][h][:],
                        wv_in[1024 * ih:1024 * (ih + 1),
                              1024 * h:1024 * (h + 1)]
                        .rearrange("(k p) d -> p k d", p=128))

            # ---------------- constants (gpsimd engine, after triggers) ---
            ident_sb = const.tile([128, 128], BF16, name="ident_sb")
            make_identity(nc, ident_sb[:])
            iota_sb = const.tile([128, BK], F32, name="iota_sb")
            nc.gpsimd.iota(
                iota_sb[:], pattern=[[-1, BK]], base=0, channel_multiplier=1,
                allow_small_or_imprecise_dtypes=True,
            )

            partials = const.tile([128, NPAIR], F32, name="partials")
            den = const.tile([128, 4], F32, name="den")
            rec = const.tile([128, 4], F32, name="rec")

            # ------- phase K: local K^T (all d_out, own q) -------
            # kst cols 0:256 -> kq0 (written immediately, AGs fire at dp3
            # and dp7); cols 256:512 are parked in SBUF (k1all) and only
            # written to kq1 after V slot 0, spacing KAG1's trigger well
            # after KAG0b's and well before VAG0a's.
            k1all = stagep.tile([128, 16, 256], BF16, tag="k1all", bufs=1,
                                name="k1all")
            for dp in range(8):
                psA = psum.tile([128, BK], F32, tag="pw", bufs=3,
                                name=f"pskA{dp}")
                psB = psum.tile([128, BK], F32, tag="pw", bufs=3,
                                name=f"pskB{dp}")
                for i, ki in enumerate(KI_ORDER):
                    wkh = wkA if ki < 8 else wkB
                    nc.tensor.matmul(
                        psA[:, 0:SB], lhsT=wkh[:, ki % 8, 256 * dp:
                                               256 * dp + 128],
                        rhs=xq_sb[:, ki, :],
                        start=(i == 0), stop=(i == 15),
                    )
                    nc.tensor.matmul(
                        psB[:, 0:SB], lhsT=wkh[:, ki % 8, 256 * dp + 128:
                                               256 * dp + 256],
                        rhs=xq_sb[:, ki, :],
                        start=(i == 0), stop=(i == 15),
                    )
                for half, ps in ((0, psA), (1, psB)):
                    dt = 2 * dp + half
                    kst = stagep.tile([128, 256], BF16, tag="kst", bufs=6,
                                      name=f"kst{dt}")
                    nc.vector.tensor_copy(kst[:], ps[:, 0:256])
                    nc.vector.tensor_copy(k1all[:, dt, :], ps[:, 256:512])
                    nc.sync.dma_start(
                        kq[0][dt // 8][128 * (dt % 8):
                                       128 * (dt % 8) + 128, :],
                        kst[:])
                if dp in (3, 7):
                    H = dp // 4
                    nc.gpsimd.collective_compute(
                        "AllGather", ALU.bypass, replica_groups=RG,
                        ins=[kq[0][H].opt()], outs=[kgx[0][H].opt()],
                    )

            def kq1_write(H):
                for dt in range(8 * H, 8 * H + 8):
                    nc.sync.dma_start(
                        kq[1][H][128 * (dt % 8):128 * (dt % 8) + 128, :],
                        k1all[:, dt, :])
                nc.gpsimd.collective_compute(
                    "AllGather", ALU.bypass, replica_groups=RG,
                    ins=[kq[1][H].opt()], outs=[kgx[1][H].opt()],
                )

            # ------- phase V: natural layout, slots 0..3 ------
            # h=1 vq writes are DEFERRED two phases to ladder the V-gather
            # triggers ~17us apart (see kq/kg comment).
            def vq_write(st, h, vst):
                nc.sync.dma_start(
                    vq_in[st // 2][h]
                    .rearrange("(t p) d -> p t d", p=128)[:, st % 2, :],
                    vst[:])

            def v_slot(st, defer):
                deferred = []
                for h in range(2):
                    ps = psum.tile([128, BK], F32, tag="pw", bufs=3,
                                   name=f"psv{st}_{h}")
                    for ki in range(16):
                        for n2 in range(2):
                            nc.tensor.matmul(
                                ps[:, 512 * n2:512 * (n2 + 1)],
                                lhsT=xq_sb[:, ki, 128 * st:128 * (st + 1)],
                                rhs=wvq[ki // 8][h][
                                    :, ki % 8, 512 * n2:512 * (n2 + 1)],
                                start=(ki == 0), stop=(ki == 15),
                                skip_group_check=True,
                            )
                    vst = stagep.tile([128, BK], BF16, tag="vst", bufs=6,
                                      name=f"vst{st}_{h}")
                    nc.vector.tensor_copy(vst[:], ps[:])
                    if h in defer:
                        deferred.append(vst)
                    else:
                        vq_write(st, h, vst)
                return deferred

            # Chain order = consumption order; each op's input completes at
            # a distinct compute-gated point ~15us after the previous one:
            #   KAG0a (K dp3), KAG0b (K dp7), VAG0a (Vc1 st1-h0),
            #   VAG0b (after st2), KAG1a (after st3), KAG1b (Q dp1),
            #   VAG1a (Q dp3), VAG1b (Q dp5).
            vd0 = v_slot(0, defer={1})          # st0: h1 deferred
            vd1 = v_slot(1, defer={1})          # st1: h1 deferred
            nc.gpsimd.collective_compute(
                "AllGather", ALU.bypass, replica_groups=RG,
                ins=[vq_in[0][0].opt()], outs=[vgh[0][0].opt()],
            )
            vd2 = v_slot(2, defer={0, 1})       # st2: both deferred
            vq_write(0, 1, vd0[0])
            vq_write(1, 1, vd1[0])
            nc.gpsimd.collective_compute(
                "AllGather", ALU.bypass, replica_groups=RG,
                ins=[vq_in[0][1].opt()], outs=[vgh[0][1].opt()],
            )
            vd3 = v_slot(3, defer={0, 1})       # st3: both deferred
            kq1_write(0)

            # kt tiles: ktA = d_out rows 0..1023 (score ki 0..7), ktB =
            # rows 1024..2047, from kg0[H] (blocks 0/1) or kg1 (blocks 2/3).
            # All 8 tiles of a block share slot parity: one 128-col stripe.
            def emit_kt(B):
                ktA = wvkt.tile([128, 8, BK], BF16, tag="wvkt",
                                name=f"ktA{B}")
                ktB = wvkt.tile([128, 8, BK], BF16, tag="wvkt",
                                name=f"ktB{B}")
                g = B // 2
                coff = 128 * (B % 2)
                # Attention loads go on sync+gpsimd ONLY: scalar-engine DMA
                # triggers would head-of-line-block the pex activations
                # behind their AG waits (measured 57us pipeline collapse).
                for H, kth, eng in ((0, ktA, nc.sync), (1, ktB, nc.gpsimd)):
                    for j in range(8):
                        t = 8 * B + j
                        c, _s = tile_owner_slot(t)
                        eng.dma_start(
                            kth[:, :, 128 * j:128 * (j + 1)],
                            kgx[g][H][BK * c:BK * (c + 1), coff:coff + 128]
                            .rearrange("(k p) q -> p k q", p=128),
                        )
                return ktA, ktB

            # blocks 0/1 kt tiles rotate into the wv buffers right here:
            # their loads run during the Q projection.
            kt0 = emit_kt(0)
            kt1 = emit_kt(1)

            # Wq^T halves rotate into Wk^T's buffers (dep: K matmuls done).
            wqA = wkq.tile([128, 8, D], BF16, tag="wh", name="wqA")
            wqB = wkq.tile([128, 8, D], BF16, tag="wh", name="wqB")
            nc.scalar.dma_start(
                wqA[:], wq_in[0:1024, :].rearrange("(k p) d -> p k d", p=128))
            nc.scalar.dma_start(
                wqB[:], wq_in[1024:2048, :].rearrange("(k p) d -> p k d",
                                                      p=128))

            # ------- phase Q: local Q^T projection ------
            for dp in range(8):
                psA = psum.tile([128, BK], F32, tag="pw", bufs=3,
                                name=f"psqA{dp}")
                psB = psum.tile([128, BK], F32, tag="pw", bufs=3,
                                name=f"psqB{dp}")
                for ki in range(16):
                    wqh = wqA if ki < 8 else wqB
                    nc.tensor.matmul(
                        psA[:, 0:SB], lhsT=wqh[:, ki % 8, 256 * dp:
                                               256 * dp + 128],
                        rhs=xq_sb[:, ki, :],
                        start=(ki == 0), stop=(ki == 15),
                    )
                    nc.tensor.matmul(
                        psB[:, 0:SB], lhsT=wqh[:, ki % 8, 256 * dp + 128:
                                               256 * dp + 256],
                        rhs=xq_sb[:, ki, :],
                        start=(ki == 0), stop=(ki == 15),
                    )
                nc.vector.tensor_copy(qt_sb[:, 2 * dp, :], psA[:, 0:SB])
                nc.vector.tensor_copy(qt_sb[:, 2 * dp + 1, :],
                                      psB[:, 0:SB])
                if dp == 1:
                    kq1_write(1)
                elif dp == 3:
                    vq_write(2, 0, vd2[0])
                    vq_write(3, 0, vd3[0])
                    nc.gpsimd.collective_compute(
                        "AllGather", ALU.bypass, replica_groups=RG,
                        ins=[vq_in[1][0].opt()], outs=[vgh[1][0].opt()],
                    )
                elif dp == 5:
                    vq_write(2, 1, vd2[1])
                    vq_write(3, 1, vd3[1])
                    nc.gpsimd.collective_compute(
                        "AllGather", ALU.bypass, replica_groups=RG,
                        ins=[vq_in[1][1].opt()], outs=[vgh[1][1].opt()],
                    )
            qes.close()

            # ---------------- attention (software-pipelined) ----------------
            accp = es.enter_context(tc.tile_pool(name="accp", bufs=1))
            vtl = es.enter_context(tc.tile_pool(name="vtl", bufs=4))
            mkl = es.enter_context(tc.tile_pool(name="mkl", bufs=3))
            pwork = es.enter_context(tc.tile_pool(name="pwork", bufs=2))

            acc = [accp.tile([128, D], F32, name=f"acc{t}") for t in range(4)]

            # Dropout masks load lazily: 3 upfront, then pair i+3's mask is
            # emitted right after pair i's pm frees its ring slot, so the
            # scalar-queue trigger NEVER waits (a waiting mask trigger
            # head-of-line-blocks the pex activations: measured 30us stall).
            PAIRLIST = [(B, s) for B in range(NBIG) for s in range(B, 4)]
            mk_all = {}

            def emit_mask(i):
                if i >= len(PAIRLIST):
                    return
                B, slot = PAIRLIST[i]
                mk = mkl.tile([128, BK], BF16, tag="mk",
                              name=f"mk{B}_{slot}")
                nc.scalar.dma_start(
                    mk[:],
                    mask_in[128 * slot:128 * (slot + 1),
                            BK * B:BK * (B + 1)],
                )
                mk_all[(B, slot)] = mk

            for i in range(3):
                emit_mask(i)

            def emit_vt_half(B, h, eng):
                vt = vtl.tile([128, 8, BK], BF16, tag="vt",
                              name=f"vt{'AB'[h]}{B}")
                for j in range(8):
                    r0 = VROW2[8 * B + j]
                    eng.dma_start(vt[:, j, :], vgh[B // 2][h][r0:r0 + 128, :])
                return vt

            def normalize_slot(slot):
                obf = pwork.tile([128, D], BF16, tag="obf", bufs=1,
                                 name=f"obf{slot}")
                nc.vector.tensor_reduce(
                    den[:, slot:slot + 1],
                    partials[:, PBASE[slot]:PBASE[slot] + KBMAX[slot]],
                    axis=mybir.AxisListType.X, op=ALU.add,
                )
                nc.vector.reciprocal(rec[:, slot:slot + 1],
                                     den[:, slot:slot + 1])
                nc.vector.tensor_scalar_mul(
                    obf[:], acc[slot][:], rec[:, slot:slot + 1])
                nc.scalar.dma_start(
                    out_ext[128 * slot:128 * (slot + 1), :], obf[:])

            def tp_stage(st):
                pm, vtA, vtB, B, slot = st
                pmt = pwork.tile([128, 8, 128], BF16, tag="pmt", bufs=3,
                                 name=f"pmt{B}_{slot}")
                for j in range(8):
                    tp = psum.tile([128, 128], BF16, tag="tp", bufs=2,
                                   name=f"tp{B}_{slot}_{j}")
                    nc.tensor.matmul(
                        tp[:], lhsT=pm[:, 128 * j:128 * (j + 1)],
                        rhs=ident_sb[:], is_transpose=True,
                        skip_group_check=True)
                    nc.scalar.copy(pmt[:, j, :], tp[:])
                return pmt

            def av_stage(st, pmt):
                pm, vtA, vtB, B, slot = st
                for h, vt in ((0, vtA), (1, vtB)):
                    av = psum.tile([128, BK], F32, tag="pw", bufs=3,
                                   name=f"av{B}_{slot}_{h}")
                    for j in range(8):
                        for n2 in range(2):
                            nc.tensor.matmul(
                                av[:, 512 * n2:512 * (n2 + 1)],
                                lhsT=pmt[:, j, :],
                                rhs=vt[:, j, 512 * n2:512 * (n2 + 1)],
                                start=(j == 0), stop=(j == 7),
                                skip_group_check=True,
                            )
                    if B == 0:
                        nc.vector.tensor_copy(
                            acc[slot][:, BK * h:BK * (h + 1)], av[:])
                    else:
                        nc.vector.scalar_tensor_tensor(
                            out=acc[slot][:, BK * h:BK * (h + 1)],
                            in0=av[:], scalar=1.0,
                            in1=acc[slot][:, BK * h:BK * (h + 1)],
                            op0=ALU.mult, op1=ALU.add,
                        )

            # software pipeline state: pair p's P-transposes run during
            # pair p+1's scores; its attn@V runs after pair p+2's scores
            # (the 2-pair lag lets the vt loads finish behind the Q-end
            # SBUF release without stalling the PE).
            state = {"prev": None, "prev_pmt": None, "old": None,
                     "old_pmt": None}

            def retire_old():
                if state["old"] is not None:
                    av_stage(state["old"], state["old_pmt"])
                    oB, oslot = state["old"][3], state["old"][4]
                    if oB == oslot:
                        # slot oslot's accumulation is complete (its
                        # diagonal block was its last): normalize and
                        # write it out now, hidden under later pairs.
                        normalize_slot(oslot)

            def emit_pairs(B, ktA, ktB, vtA, vtB):
                for slot in range(B, 4):
                    p = PBASE[slot] + B
                    mk = mk_all[(B, slot)]
                    sc = psum.tile([128, BK], F32, tag="pw", bufs=3,
                                   name=f"sc{B}_{slot}")
                    for ki in range(16):
                        if ki == 8 and state["prev"] is not None:
                            # interleave prev pair's P-transposes here so
                            # the pmt copies finish before its attn@V
                            state["prev_pmt"] = tp_stage(state["prev"])
                        kth = ktA if ki < 8 else ktB
                        for n2 in range(2):
                            nc.tensor.matmul(
                                sc[:, 512 * n2:512 * (n2 + 1)],
                                lhsT=qt_sb[:, ki, 128 * slot:128 * (slot + 1)],
                                rhs=kth[:, ki % 8, 512 * n2:512 * (n2 + 1)],
                                start=(ki == 0), stop=(ki == 15),
                                skip_group_check=True,
                            )
                    pex = pwork.tile([128, BK], BF16, tag="pex", bufs=1,
                                     name=f"pex{B}_{slot}")
                    nc.scalar.activation(pex[:], sc[:], AFT.Exp, scale=SCALE)
                    pcs = pwork.tile([128, BK], BF16, tag="pcs", bufs=1,
                                     name=f"pcs{B}_{slot}")
                    nc.vector.scalar_tensor_tensor(
                        out=pcs[:], in0=iota_sb[:],
                        scalar=sched_sb[:, p:p + 1], in1=pex[:],
                        op0=ALU.is_ge, op1=ALU.mult,
                        accum_out=partials[:, p:p + 1],
                    )
                    pm = pwork.tile([128, BK], BF16, tag="pm", bufs=3,
                                    name=f"pm{B}_{slot}")
                    nc.vector.tensor_mul(pm[:], pcs[:], mk[:])
                    emit_mask(PAIRLIST.index((B, slot)) + 3)
                    retire_old()
                    state["old"] = state["prev"]
                    state["old_pmt"] = state["prev_pmt"]
                    state["prev"] = (pm, vtA, vtB, B, slot)

            # group 1: blocks 0 and 1 (kt tiles already loading since the
            # V phase ended; vt tiles load from attention start).
            vtA0 = emit_vt_half(0, 0, nc.sync)
            vtB0 = emit_vt_half(0, 1, nc.gpsimd)
            vtA1 = emit_vt_half(1, 0, nc.sync)
            vtB1 = emit_vt_half(1, 1, nc.gpsimd)

            # a short filler bridges the first pair's exp->mask->transpose
            # pipeline fill so the PE HAM throttle never re-engages.
            for w in range(12):
                tpw = psum.tile([128, 128], BF16, tag="tp", bufs=2,
                                name=f"warm{w}")
                nc.tensor.matmul(
                    tpw[:], lhsT=ident_sb[:], rhs=ident_sb[:],
                    is_transpose=True, skip_group_check=True)

            emit_pairs(0, kt0[0], kt0[1], vtA0, vtB0)
            emit_pairs(1, kt1[0], kt1[1], vtA1, vtB1)

            # group 2: blocks 2 and 3.  vt loads go on gpsimd — safe here
            # because no CC trigger is emitted after them.
            kt2 = emit_kt(2)
            kt3 = emit_kt(3)
            vtA2 = emit_vt_half(2, 0, nc.sync)
            vtA3 = emit_vt_half(3, 0, nc.sync)
            # vtB2/vtB3 depend on the LAST gather (VAG1b): split them
            # across both queues so the final attn@V tail isn't serialized
            # behind a single 22us gpsimd drain.
            vtB2 = emit_vt_half(2, 1, nc.gpsimd)
            vtB3 = emit_vt_half(3, 1, nc.sync)
            emit_pairs(2, kt2[0], kt2[1], vtA2, vtB2)
            emit_pairs(3, kt3[0], kt3[1], vtA3, vtB3)

            retire_old()
            state["old"] = state["prev"]
            state["old_pmt"] = tp_stage(state["prev"])
            retire_old()   # retires (3,3), which also normalizes slot 3

    nc.compile()
    return nc


_NC_CACHE = None


def _get_nc():
    global _NC_CACHE
    if _NC_CACHE is None:
        _NC_CACHE = build()
    return _NC_CACHE


def make_in_maps(x, Wq, Wk, Wv, drop_mask):
    bf = ml_dtypes.bfloat16
    x = np.asarray(x, dtype=np.float32)
    Wq = np.asarray(Wq, dtype=np.float32)
    Wk = np.asarray(Wk, dtype=np.float32)
    Wv = np.asarray(Wv, dtype=np.float32)
    drop_mask = np.asarray(drop_mask, dtype=np.float32)

    xT = np.ascontiguousarray(x.T).astype(bf)           # [D, S]
    wqT = np.ascontiguousarray(Wq.T.astype(bf))         # [D, D]
    wvT = np.ascontiguousarray(Wv.T.astype(bf))         # [D, D]
    wkT = np.ascontiguousarray(Wk.T.astype(bf))         # [D, D]
    mask_bf = drop_mask.astype(bf)

    in_maps = []
    for c in range(NC):
        tl = owned_tiles(c)
        thr = np.array(
            [1024.0 * B - 128.0 * tl[slot]
             for slot in range(4) for B in range(KBMAX[slot])],
            dtype=np.float32,
        )
        in_maps.append({
            "xq": np.ascontiguousarray(
                np.concatenate([xT[:, 128 * t:128 * (t + 1)] for t in tl],
                               axis=1)),
            "wqT": wqT,
            "wvT": wvT,
            "wkT": wkT,
            "drop_mask": np.ascontiguousarray(
                np.concatenate(
                    [mask_bf[128 * t:128 * (t + 1)] for t in tl], axis=0)),
            "sched": np.ascontiguousarray(np.tile(thr[None, :], (128, 1))),
        })
    return in_maps


def assemble(results):
    full = np.zeros((S, D), dtype=np.float32)
    for c in range(NC):
        o = np.asarray(results[c]["out"], dtype=np.float32)
        for slot, t in enumerate(owned_tiles(c)):
            full[128 * t:128 * (t + 1)] = o[128 * slot:128 * (slot + 1)]
    return full


def kernel(x, Wq, Wk, Wv, drop_mask):
    nc = _get_nc()
    in_maps = make_in_maps(x, Wq, Wk, Wv, drop_mask)
    res = bass_utils.run_bass_kernel_spmd(nc, in_maps, core_ids=list(range(NC)))
    return assemble(res.results)


def kernel_profiled(x, Wq, Wk, Wv, drop_mask):
    """Like kernel(), but captures an NTFF profile; returns (out, exec_time_ns,
    trace_path)."""
    nc = _get_nc()
    in_maps = make_in_maps(x, Wq, Wk, Wv, drop_mask)
    res = bass_utils.run_bass_kernel_spmd(
        nc, in_maps, core_ids=list(range(NC)), trace=True)
    trace_path = None
    if res.instructions_and_trace is not None:
        trace_path = res.instructions_and_trace[1]
    return assemble(res.results), res.exec_time_ns, trace_path
